# revision 6
# baseline (speedup 1.0000x reference)
"""CrossAttentionFusion Trainium2 kernel: fp8-e4m3 DoubleRow attention.

Problem (per batch element b of 4, C=128 channels, N=4096 tokens):
    Q1 = wq1@hsi+bq1; K1 = wk1@msi+bk1; V1 = wv1@msi+bv1   (1x1 convs)
    Q2 = wq2@msi+bq2; K2 = wk2@hsi+bk2; V2 = wv2@hsi+bv2
    out1 = attn(Q1,K1,V1); out2 = attn(Q2,K2,V2)           (softmax over keys)
    g = sigmoid(wg@[hsi;msi]+bg)
    out = wp@(g*out1 + (1-g)*out2) + bp

Sharding: 8 cores = (b, query-half); the host rotates the token axis per core
so each core's queries are columns [0, NQ) and the SPMD program is offset-free.

Dataflow (every piece validated in micro_test*.py + exact numpy emulation;
measured end-to-end rel_max 1.45e-2 against the fp32 reference, gate 2e-2):
  - Empirical PE rate law (from traces): ~1 output column/cycle at 2.4GHz for
    every dtype; DoubleRow fp8 doubles CONTRACTION per column, not column
    rate. So wins come from cutting output columns, not switching dtypes.
  - pt = exp(scale*s - SH) stored as fp8-e4m3 (SH=2 keeps exp <= 105 < 240).
    Producers alternate per key tile: ACT tiles use the Exp table with e4
    output; DVE tiles use a uint8-domain Schraudolph (single tensor_scalar;
    the f32->u8 convert rounds-to-nearest and saturates [0,255], and
    bits = A8*(scale*s - SH) + B8 are raw e4m3 bit patterns).
  - PV matmul: fp8 DoubleRow contracts TWO key tiles per pass (weights =
    adjacent V^T tiles, moving = the adjacent [pt_2k|pt_2k+1] pair), halving
    PV output columns vs fp32r. V is single e4m3 (err ~2% RMS, averaged out).
  - softmax denominator: same DoubleRow trick against an e4 ones [128,2,128]
    block, halving den columns and eliminating all DVE pair-add chains.
  - scores stay fp32r (fp8 QK fails the precision gate); x and the conv/gate
    weights ride fp16 (halves input DMA; ~1e-4 noise).
  - PSUM layout: 4-deep ring of [128,512] score half-slots (the exp
    producer's ~1.0us round trip fits inside the ~1.5us half-slot reuse
    distance, so the PE never waits) + 2x2 accumulator slots for PV/den.
  - pv_den lags scores by 5 tiles so a chunk's first PV lands after the
    previous chunk's normalize has freed the accumulator slots.
  - V biases fold into u = (1+t')bv1 + (1-t')bv2 (per-partition affine of
    tanh) and ride the projection PSUM accumulation; the gated fuse-add does
    too, so the tail chain is rec -> normalize -> o2-gate -> project.
  - input DMA: fp16 pieces on both HWDGE rings ordered by first use, with a
    tiny wk1-only weight DMA and a half-size first xm piece so the first conv
    dispatches as early as the descriptor-generation latency allows.
Engine split: ACT = exp tiles + V evictions + tanh + u + proj evictions;
DVE = Schraudolph tiles + K/Q evictions + reciprocal + normalize + fusion
muls; GpSimd (no PSUM access, ~8x slower than modeled on tensor_scalar) =
only the early gate transforms.
"""

import sys

if "/opt/trn_rl_repo" not in sys.path:
    sys.path.insert(0, "/opt/trn_rl_repo")

from contextlib import ExitStack

import numpy as np

import concourse.bacc as bacc
import concourse.bass as bass  # noqa: F401
import concourse.tile as tile
from concourse import mybir

F32 = mybir.dt.float32
F32R = mybir.dt.float32r
F16 = mybir.dt.float16
E4 = mybir.dt.float8e4
U8 = mybir.dt.uint8
C = 128
N_TOK = 4096
NQ = 2048
FD = 512   # matmul moving-operand max for 4-byte dtypes
CH = 1024  # query-chunk width (PSUM accumulator width)
SCALE = 1.0 / float(np.sqrt(np.float32(C)))
SH = 2.0                       # softmax shift: pt = exp(scale*s - SH)
A8 = 8.0 / float(np.log(2.0))  # e4m3 bits per nat
B8 = 56.0 - 0.3434             # e4m3 bias-7 bit offset, RMS-centered
def act_tile(m, mt):
    """pt-producer pick: ACT exp vs DVE Schraudolph. First 3 tiles go to ACT
    (the PE pipeline is still filling, so a serial ACT run is free), then the
    engines strictly alternate — no same-engine run mid-chunk means the
    2-deep PSUM score ring never waits on a busy producer. 18/32 per chunk
    on ACT balances total ACT vs DVE load."""
    return m < 3 or m % 2 == 1

# conv/gate/V weights and x ride in fp16 (~1e-4 relative noise, invisible
# next to the fp8 pt quantization); wpTs stays fp32r (its moving op is f32r).
# Pack A = attention-1 + gate weights (heads its DMA ring), pack B = attn-2.
WA = [("wk1T", 128), ("wv1T", 128), ("wq1T", 128), ("wgaT", 128), ("wgbT", 128)]
WB = [("wk2T", 128), ("wv2T", 128), ("wq2T", 128)]
BIAS_NAMES = ["bq1", "bk1", "bq2", "bk2", "bvd", "bvs", "bgh", "bp", "bsh"]


def _r(ap):
    return ap.bitcast(F32R)


def build_program(n_tok=N_TOK, nq=NQ, ch=CH, fd=FD):
    mt = n_tok // 128   # key tiles
    nch = nq // ch      # query chunks per attention
    spc = ch // fd      # matmul slices per chunk

    nc = bacc.Bacc("TRN2", target_bir_lowering=False, debug=False)
    din = {}
    for name in ["x_h", "x_m"]:
        din[name] = nc.dram_tensor(name, [C, n_tok], F16, kind="ExternalInput").ap()
    nwa = sum(w for _, w in WA)
    nwb = sum(w for _, w in WB)
    din["wpackA"] = nc.dram_tensor("wpackA", [C, nwa], F16, kind="ExternalInput").ap()
    din["wpackB"] = nc.dram_tensor("wpackB", [C, nwb], F16, kind="ExternalInput").ap()
    din["wp32"] = nc.dram_tensor("wp32", [C, C], F32, kind="ExternalInput").ap()
    din["bpack"] = nc.dram_tensor(
        "bpack", [C, len(BIAS_NAMES)], F32, kind="ExternalInput"
    ).ap()
    out_d = nc.dram_tensor("out", [C, nq], F32, kind="ExternalOutput").ap()

    with ExitStack() as ctx:
        tc = ctx.enter_context(tile.TileContext(nc))
        const = ctx.enter_context(tc.tile_pool(name="const", bufs=1))
        big = ctx.enter_context(tc.tile_pool(name="big", bufs=1))
        ppool = ctx.enter_context(tc.tile_pool(name="ppool", bufs=4))
        rpool = ctx.enter_context(tc.tile_pool(name="rpool", bufs=2))
        stpool = ctx.enter_context(tc.tile_pool(name="stpool", bufs=2))
        ps_pool = ctx.enter_context(tc.tile_pool(name="ps", bufs=4, space="PSUM"))
        pacc_pool = ctx.enter_context(tc.tile_pool(name="pacc", bufs=2, space="PSUM"))

        # ---- inputs: 1024-col fp16 pieces on both rings, ordered by first
        # use: xm piece 0 + bias pack head the sync ring, weight pack A heads
        # the scalar ring, so the first conv starts as early as possible
        xh_sb = big.tile([C, n_tok], F16, name="xh")
        xm_sb = big.tile([C, n_tok], F16, name="xm")
        wpackA_sb = const.tile([C, nwa], F16, name="wpackA")
        wpackB_sb = const.tile([C, nwb], F16, name="wpackB")
        wp32_sb = const.tile([C, C], F32R, name="wp32")
        bpack_sb = const.tile([C, len(BIAS_NAMES)], F32, name="bpack")
        dch = min(1024, n_tok)
        npc = n_tok // dch
        xs = lambda i: slice(i * dch, (i + 1) * dch)
        # first piece split in half and the K1 weight sent alone: the first
        # conv matmul only needs wk1T + 512 cols of xm, so it dispatches
        # right after ~160KB of descriptors instead of ~500KB
        hp = dch // 2
        nc.scalar.dma_start(out=wpackA_sb[:, :C], in_=din["wpackA"][:, :C])
        nc.sync.dma_start(out=xm_sb[:, 0:hp], in_=din["x_m"][:, 0:hp])
        nc.sync.dma_start(out=xm_sb[:, hp:dch], in_=din["x_m"][:, hp:dch])
        nc.scalar.dma_start(out=wpackA_sb[:, C:], in_=din["wpackA"][:, C:])
        nc.sync.dma_start(out=bpack_sb[:], in_=din["bpack"][:])
        for i in range(1, npc):
            eng = nc.scalar if i % 2 == 1 else nc.sync
            eng.dma_start(out=xm_sb[:, xs(i)], in_=din["x_m"][:, xs(i)])
        for i in range(npc):
            eng = nc.sync if i % 2 == 0 else nc.scalar
            eng.dma_start(out=xh_sb[:, xs(i)], in_=din["x_h"][:, xs(i)])
        nc.sync.dma_start(out=wp32_sb[:], in_=_r(din["wp32"][:]))
        nc.scalar.dma_start(out=wpackB_sb[:], in_=din["wpackB"][:])
        w_sb = {}
        off = 0
        for name, wid in WA:
            w_sb[name] = wpackA_sb[:, off : off + wid]
            off += wid
        off = 0
        for name, wid in WB:
            w_sb[name] = wpackB_sb[:, off : off + wid]
            off += wid
        w_sb["wpTs"] = wp32_sb[:]
        b_sb = {name: bpack_sb[:, i : i + 1] for i, name in enumerate(BIAS_NAMES)}
        ones2 = const.tile([C, 2, C], E4, name="ones2")
        nc.gpsimd.memset(ones2[:], 1.0)

        K1_sb = big.tile([C, n_tok], F32R, name="K1")
        K2_sb = big.tile([C, n_tok], F32R, name="K2")
        Q1_sb = big.tile([C, nq], F32R, name="Q1")
        Q2_sb = big.tile([C, nq], F32R, name="Q2")
        # V^T e4: [tok-in-tile, pair-parity, whichV, pair-major channel col];
        # PV DoubleRow contracts the parity dim = two key tiles per pass
        vpk = big.tile([C, 2, 2, (mt // 2) * 128], E4, name="vpk")
        o1_sb = big.tile([C, nq], F32R, name="o1")
        o2_sb = big.tile([C, nq], F32R, name="o2")
        t_sb = big.tile([C, nq], F32R, name="t")
        tb_sb = big.tile([C, nq], F32R, name="tb")
        d_sb = big.tile([C, nq], F32R, name="d")

        def conv(dst_sb, wT_sb, x_sb, j, bias_sb, ei):
            # per-slice psum half-slots; evictions alternate ACT/DVE so the
            # ring drains at 2x single-engine rate during the conv phase
            for s in range(spc):
                sl = slice(j * ch + s * fd, j * ch + (s + 1) * fd)
                ps = ps_pool.tile([C, fd], F32, tag="ps", name="psc")
                nc.tensor.matmul(ps[:], wT_sb, x_sb[:, sl], start=True, stop=True)
                if (ei + s) % 2 == 0:
                    nc.scalar.activation(
                        dst_sb[:, sl], ps[:],
                        mybir.ActivationFunctionType.Identity, bias=bias_sb,
                    )
                else:
                    nc.vector.tensor_scalar_add(dst_sb[:, sl], ps[:], bias_sb)

        def vt_conv(a, x_sb, wvT_sb, g):
            # V^T (single e4) for key-tile pair 2g, 2g+1 of attention a.
            # fp16 moving operands pay no below-256-col rate penalty, so each
            # key tile is one 128-col matmul.
            ps = ps_pool.tile([C, 2, C], F32, tag="ps", name="psv")
            for u in range(2):
                j = 2 * g + u
                nc.tensor.matmul(
                    ps[:, u, :], x_sb[:, j * 128 : (j + 1) * 128], wvT_sb,
                    start=True, stop=True,
                )
            dst = vpk[:, :, a, g * 128 : (g + 1) * 128]
            if g % 2 == 0:
                nc.scalar.copy(dst, ps[:])
            else:
                nc.vector.tensor_copy(dst, ps[:])

        # ---- conv phase, ordered by input arrival (xm pieces, then xh);
        # vt pairs interleave with K convs: the K matmuls keep the PE busy
        # while vt evictions recycle the other PSUM ring
        ppj = max(1, (ch // 256))  # vt pairs per ch-wide column chunk
        def kv_phase(a, K_dst, wk, wv, x_sb, bk):
            for j in range(n_tok // ch):
                conv(K_dst, wk, x_sb, j, bk, j)
                for u in range(ppj):
                    g = j * ppj + u
                    if g < mt // 2:
                        vt_conv(a, x_sb, wv, g)
            for g in range((n_tok // ch) * ppj, mt // 2):
                vt_conv(a, x_sb, wv, g)

        kv_phase(0, K1_sb, w_sb["wk1T"], w_sb["wv1T"], xm_sb, b_sb["bk1"])
        for j in range(nq // ch):
            conv(Q1_sb, w_sb["wq1T"], xh_sb, j, b_sb["bq1"], j)
        # gate needs only xh/xm cols < nq: compute early, off the critical path
        for j in range(nq // ch):
            for s in range(spc):
                sl = slice(j * ch + s * fd, j * ch + (s + 1) * fd)
                ps = ps_pool.tile([C, fd], F32, tag="ps", name="psg")
                nc.tensor.matmul(ps[:], w_sb["wgaT"], xh_sb[:, sl], start=True, stop=False)
                nc.tensor.matmul(ps[:], w_sb["wgbT"], xm_sb[:, sl], start=False, stop=True)
                nc.scalar.activation(
                    t_sb[:, sl], ps[:],
                    mybir.ActivationFunctionType.Tanh, bias=b_sb["bgh"], scale=0.5,
                )
        kv_phase(1, K2_sb, w_sb["wk2T"], w_sb["wv2T"], xh_sb, b_sb["bk2"])
        for j in range(nq // ch):
            conv(Q2_sb, w_sb["wq2T"], xm_sb, j, b_sb["bq2"], j + 1)
        # u = (1+t')*bv1 + (1-t')*bv2 = t'*(bv1-bv2) + (bv1+bv2): the V biases
        # ride the projection as a third accumulating matmul, so no per-chunk
        # bias adds and no bias wait on the tail chain (ACT, off-path)
        u_sb = big.tile([C, nq], F32R, name="u")
        for j in range(nq // ch):
            sl = slice(j * ch, (j + 1) * ch)
            nc.scalar.activation(
                u_sb[:, sl], t_sb[:, sl],
                mybir.ActivationFunctionType.Identity,
                bias=b_sb["bvs"], scale=b_sb["bvd"],
            )
        # t <- 1+tanh, tb <- 1-tanh (fused = 0.5*[o1*(1+t') + o2*(1-t')], 0.5 in wp)
        for j in range(nq // ch):
            sl = slice(j * ch, (j + 1) * ch)
            nc.gpsimd.tensor_scalar(
                tb_sb[:, sl], t_sb[:, sl], -1.0, 1.0,
                mybir.AluOpType.mult, mybir.AluOpType.add,
            )
            nc.gpsimd.tensor_scalar_add(t_sb[:, sl], t_sb[:, sl], 1.0)

        # Deferred thunks (GpSimd/DVE/PE-proj) drained inside attention loops.
        events = []

        mA = float(SCALE * A8)
        mB = float(B8 - SH * A8)

        def attention(a, o_sb, Q_sb, K_sb, post_chunk=None):
            for cidx in range(nch):
                p_out = pacc_pool.tile([C, ch], F32, tag="acc")
                p_den = pacc_pool.tile([C, ch], F32, tag="acc")
                pts = {}

                def scores(m):
                    # per-slice psum half-slots on a 4-deep ring: the
                    # producer's ~1.0us half-op round trip now fits inside
                    # the ~1.5us half-slot reuse distance, so the PE never
                    # waits on exp
                    ksl = slice(m * 128, (m + 1) * 128)
                    if m % 2 == 0:
                        pts[m // 2] = ppool.tile([C, 2, ch], E4, tag="pt", name="pt")
                    pt_half = pts[m // 2][:, m % 2, :]
                    for s in range(spc):
                        qsl = slice(cidx * ch + s * fd, cidx * ch + (s + 1) * fd)
                        ps = ps_pool.tile([C, fd], F32, tag="ps", name="pss")
                        nc.tensor.matmul(
                            ps[:], K_sb[:, ksl], Q_sb[:, qsl], start=True, stop=True
                        )
                        ssl = slice(s * fd, (s + 1) * fd)
                        if act_tile(m, mt):
                            nc.scalar.activation(
                                pt_half[:, ssl], ps[:],
                                mybir.ActivationFunctionType.Exp,
                                bias=b_sb["bsh"], scale=SCALE,
                            )
                        else:
                            nc.vector.tensor_scalar(
                                pt_half[:, ssl].bitcast(U8), ps[:], mA, mB,
                                mybir.AluOpType.mult, mybir.AluOpType.add,
                            )

                def pv_den(k):
                    # both PV and den contract key-tile pair k in one
                    # DoubleRow pass over the same [pt_2k|pt_2k+1] moving data
                    first, last = k == 0, k == mt // 2 - 1
                    wv = vpk[:, :, a, k * 128 : (k + 1) * 128]
                    for s in range(spc):
                        ssl = slice(s * fd, (s + 1) * fd)
                        nc.tensor.matmul(
                            p_out[:, ssl], wv, pts[k][:, :, ssl],
                            start=first, stop=last,
                            perf_mode=mybir.MatmulPerfMode.DoubleRow,
                        )
                    for s in range(spc):
                        ssl = slice(s * fd, (s + 1) * fd)
                        nc.tensor.matmul(
                            p_den[:, ssl], ones2[:], pts[k][:, :, ssl],
                            start=first, stop=last,
                            perf_mode=mybir.MatmulPerfMode.DoubleRow,
                        )
                    if k >= 2:
                        pts.pop(k - 2)

                # pv_den lags scores by 5 tiles (minimum 3): the first PV
                # of a chunk then fires after the previous chunk's normalize
                # has freed the accumulator slots
                scores(0)
                scores(1)
                for m in range(2, mt):
                    scores(m)
                    if m % 2 == 1 and m >= 5:
                        pv_den((m - 5) // 2)
                    elif m % 2 == 0 and events:
                        events.pop(0)()
                for k in (mt // 2 - 2, mt // 2 - 1):
                    pv_den(k)

                rec = rpool.tile([C, ch], F32, tag="rec")
                if a == 1 and cidx == nch - 1:
                    # tail chunk: halves let the fuse+projection chain start
                    # ~0.7us earlier
                    for s_ in range(spc):
                        ssl = slice(s_ * fd, (s_ + 1) * fd)
                        osl = slice(cidx * ch + s_ * fd, cidx * ch + (s_ + 1) * fd)
                        nc.vector.reciprocal_approx_fast(rec[:, ssl], p_den[:, ssl])
                        nc.vector.tensor_mul(o_sb[:, osl], p_out[:, ssl], rec[:, ssl])
                else:
                    nc.vector.reciprocal_approx_fast(rec[:], p_den[:])
                    osl = slice(cidx * ch, (cidx + 1) * ch)
                    nc.vector.tensor_mul(o_sb[:, osl], p_out[:], rec[:])

                if post_chunk is not None:
                    events.extend(post_chunk(cidx))

        attention(0, o1_sb, Q1_sb, K1_sb)

        # o1 <- o1*(1+t') runs as soon as attn1 finishes: drains during
        # attn2's early chunks, off the tail critical path
        for c in range(nch):
            for s_ in range(spc):
                sl = slice(c * ch + s_ * fd, c * ch + (s_ + 1) * fd)

                def _ga(sl=sl):
                    nc.vector.tensor_mul(o1_sb[:, sl], o1_sb[:, sl], t_sb[:, sl])

                events.append(_ga)

        def fuse_and_project(cidx):
            # out = wpTs.T @ [o1*(1+t') + o2*(1-t') + u] + bp, with the fuse
            # add and the u bias-term riding the projection's PSUM accumulation
            thunks = []
            for s in range(spc):
                sl = slice(cidx * ch + s * fd, cidx * ch + (s + 1) * fd)

                def _gb(sl=sl):
                    nc.vector.tensor_mul(d_sb[:, sl], o2_sb[:, sl], tb_sb[:, sl])

                def _proj(sl=sl):
                    ps = ps_pool.tile([C, fd], F32, tag="ps", name="psp")
                    nc.tensor.matmul(
                        ps[:], w_sb["wpTs"], o1_sb[:, sl], start=True, stop=False
                    )
                    nc.tensor.matmul(
                        ps[:], w_sb["wpTs"], d_sb[:, sl], start=False, stop=False
                    )
                    nc.tensor.matmul(
                        ps[:], w_sb["wpTs"], u_sb[:, sl], start=False, stop=True
                    )
                    st = stpool.tile([C, fd], F32, tag="st")
                    hq = fd // 2
                    for q_ in range(2):
                        qs = slice(q_ * hq, (q_ + 1) * hq)
                        nc.scalar.activation(
                            st[:, qs], ps[:, qs],
                            mybir.ActivationFunctionType.Identity, bias=b_sb["bp"],
                        )
                        osl = slice(sl.start + q_ * hq, sl.start + (q_ + 1) * hq)
                        nc.sync.dma_start(out=out_d[:, osl], in_=st[:, qs])

                thunks += [_gb, _proj]
            return thunks

        attention(1, o2_sb, Q2_sb, K2_sb, post_chunk=fuse_and_project)
        while events:
            events.pop(0)()

    nc.compile()
    return nc


def make_in_maps(hsi, msi, weights, n_cores=8):
    """Host-side sharding: core i handles (b=i//2, half=i%2); token axis rotated
    so the core's queries are columns [0, NQ)."""
    B = hsi.shape[0]
    hsi = np.ascontiguousarray(hsi.reshape(B, C, N_TOK), dtype=np.float16)
    msi = np.ascontiguousarray(msi.reshape(B, C, N_TOK), dtype=np.float16)
    in_maps = []
    for core in range(n_cores):
        b, h = core // 2, core % 2
        if h == 0:
            x_h, x_m = hsi[b], msi[b]
        else:
            x_h = np.concatenate([hsi[b][:, NQ:], hsi[b][:, :NQ]], axis=1)
            x_m = np.concatenate([msi[b][:, NQ:], msi[b][:, :NQ]], axis=1)
        m = {"x_h": np.ascontiguousarray(x_h), "x_m": np.ascontiguousarray(x_m)}
        m.update(weights)
        in_maps.append(m)
    return in_maps


def make_weight_map(
    wq1, bq1, wk1, bk1, wv1, bv1, wq2, bq2, wk2, bk2, wv2, bv2, wg, bg, wp, bp
):
    f = np.float32
    col = lambda v: np.ascontiguousarray(np.asarray(v, f).reshape(C, 1))
    tr = lambda w: np.ascontiguousarray(np.asarray(w, f).T)
    w = {
        "wq1T": tr(wq1), "wk1T": tr(wk1), "wq2T": tr(wq2), "wk2T": tr(wk2),
        "wgaT": tr(np.asarray(wg, f)[:, :C]),
        "wgbT": tr(np.asarray(wg, f)[:, C:]),
        "wv1T": tr(wv1),
        "wv2T": tr(wv2),
    }
    bv1a = np.asarray(bv1, f)
    bv2a = np.asarray(bv2, f)
    b = {
        "bq1": col(bq1), "bk1": col(bk1), "bq2": col(bq2), "bk2": col(bk2),
        "bvd": col(bv1a - bv2a), "bvs": col(bv1a + bv2a),
        "bgh": col(0.5 * np.asarray(bg, f)),
        "bp": col(bp), "bsh": col(np.full(C, -SH, f)),
    }
    wpackA = np.concatenate([w[n] for n, _ in WA], axis=1)
    wpackB = np.concatenate([w[n] for n, _ in WB], axis=1)
    bpack = np.concatenate([b[n] for n in BIAS_NAMES], axis=1)
    return {
        "wpackA": np.ascontiguousarray(wpackA.astype(np.float16)),
        "wpackB": np.ascontiguousarray(wpackB.astype(np.float16)),
        "wp32": np.ascontiguousarray(tr(0.5 * np.asarray(wp, f))),
        "bpack": np.ascontiguousarray(bpack),
    }


_NC_CACHE = {}


def _get_program():
    if "nc" not in _NC_CACHE:
        _NC_CACHE["nc"] = build_program()
    return _NC_CACHE["nc"]


def run_on_cores(in_maps, trace=False, **kwargs):
    from concourse.bass_utils import run_bass_kernel_spmd

    nc = _get_program()
    return run_bass_kernel_spmd(
        nc, in_maps, core_ids=list(range(len(in_maps))), trace=trace, **kwargs
    )


def kernel(
    hsi, msi, wq1, bq1, wk1, bk1, wv1, bv1, wq2, bq2, wk2, bk2, wv2, bv2,
    wg, bg, wp, bp,
):
    B, _, H, W = hsi.shape
    weights = make_weight_map(
        wq1, bq1, wk1, bk1, wv1, bv1, wq2, bq2, wk2, bk2, wv2, bv2, wg, bg, wp, bp
    )
    in_maps = make_in_maps(np.asarray(hsi), np.asarray(msi), weights)
    res = run_on_cores(in_maps)
    out = np.zeros((B, C, N_TOK), dtype=np.float32)
    for core in range(8):
        b, h = core // 2, core % 2
        out[b][:, h * NQ : (h + 1) * NQ] = res.results[core]["out"]
    return out.reshape(B, C, H, W)


# revision 7
# speedup vs baseline: 1.1962x; 1.1962x over previous
"""CrossAttentionFusion Trainium2 kernel: fp8-e4m3 DoubleRow attention.

Problem (per batch element b of 4, C=128 channels, N=4096 tokens):
    Q1 = wq1@hsi+bq1; K1 = wk1@msi+bk1; V1 = wv1@msi+bv1   (1x1 convs)
    Q2 = wq2@msi+bq2; K2 = wk2@hsi+bk2; V2 = wv2@hsi+bv2
    out1 = attn(Q1,K1,V1); out2 = attn(Q2,K2,V2)           (softmax over keys)
    g = sigmoid(wg@[hsi;msi]+bg)
    out = wp@(g*out1 + (1-g)*out2) + bp

Sharding: 8 cores = (b, query-half); the host rotates the token axis per core
so each core's queries are columns [0, NQ) and the SPMD program is offset-free.

Dataflow (every piece validated in micro_test*.py + exact numpy emulation;
measured end-to-end rel_max 1.45e-2 against the fp32 reference, gate 2e-2):
  - Empirical PE rate law (from traces): ~1 output column/cycle at 2.4GHz for
    every dtype; DoubleRow fp8 doubles CONTRACTION per column, not column
    rate. So wins come from cutting output columns, not switching dtypes.
  - pt = exp(scale*s - SH) stored as fp8-e4m3 (SH=2 keeps exp <= 105 < 240).
    Producers alternate per key tile: ACT tiles use the Exp table with e4
    output; DVE tiles use a uint8-domain Schraudolph (single tensor_scalar;
    the f32->u8 convert rounds-to-nearest and saturates [0,255], and
    bits = A8*(scale*s - SH) + B8 are raw e4m3 bit patterns).
  - PV matmul: fp8 DoubleRow contracts TWO key tiles per pass (weights =
    adjacent V^T tiles, moving = the adjacent [pt_2k|pt_2k+1] pair), halving
    PV output columns vs fp32r. V is single e4m3 (err ~2% RMS, averaged out).
  - softmax denominator: same DoubleRow trick against an e4 ones [128,2,128]
    block, halving den columns and eliminating all DVE pair-add chains.
  - scores stay fp32r (fp8 QK fails the precision gate); x and the conv/gate
    weights ride fp16 (halves input DMA; ~1e-4 noise).
  - PSUM layout: 4-deep ring of [128,512] score half-slots (the exp
    producer's ~1.0us round trip fits inside the ~1.5us half-slot reuse
    distance, so the PE never waits) + 2x2 accumulator slots for PV/den.
  - pv_den lags scores by 5 tiles so a chunk's first PV lands after the
    previous chunk's normalize has freed the accumulator slots.
  - V biases fold into u = (1+t')bv1 + (1-t')bv2 (per-partition affine of
    tanh) and ride the projection PSUM accumulation; the gated fuse-add does
    too, so the tail chain is rec -> normalize -> o2-gate -> project.
  - input DMA: fp16 pieces on both HWDGE rings ordered by first use, with a
    tiny wk1-only weight DMA and a half-size first xm piece so the first conv
    dispatches as early as the descriptor-generation latency allows.
Engine split: ACT = exp tiles + V evictions + tanh + u + proj evictions;
DVE = Schraudolph tiles + K/Q evictions + reciprocal + normalize + fusion
muls; GpSimd (no PSUM access, ~8x slower than modeled on tensor_scalar) =
only the early gate transforms.
"""

import sys

if "/opt/trn_rl_repo" not in sys.path:
    sys.path.insert(0, "/opt/trn_rl_repo")

from contextlib import ExitStack

import numpy as np

import concourse.bacc as bacc
import concourse.bass as bass  # noqa: F401
import concourse.tile as tile
from concourse import mybir

F32 = mybir.dt.float32
F32R = mybir.dt.float32r
F16 = mybir.dt.float16
E4 = mybir.dt.float8e4
U8 = mybir.dt.uint8
C = 128
N_TOK = 4096
NQ = 2048
FD = 512   # matmul moving-operand max for 4-byte dtypes
CH = 1024  # query-chunk width (PSUM accumulator width)
SCALE = 1.0 / float(np.sqrt(np.float32(C)))
SH = 2.0                       # softmax shift: pt = exp(scale*s - SH)
A8 = 8.0 / float(np.log(2.0))  # e4m3 bits per nat
B8 = 56.0 - 0.3434             # e4m3 bias-7 bit offset, RMS-centered
def act_tile(m, mt):
    """pt-producer pick: ACT exp vs DVE Schraudolph. First 3 tiles go to ACT
    (the PE pipeline is still filling, so a serial ACT run is free), then the
    engines strictly alternate — no same-engine run mid-chunk means the
    2-deep PSUM score ring never waits on a busy producer. 18/32 per chunk
    on ACT balances total ACT vs DVE load."""
    return m < 3 or m % 2 == 1

# conv/gate/V weights and x ride in fp16 (~1e-4 relative noise, invisible
# next to the fp8 pt quantization); wpTs stays fp32r (its moving op is f32r).
# Pack A = attention-1 + gate weights (heads its DMA ring), pack B = attn-2.
WA = [("wk1T", 128), ("wv1T", 128), ("wq1T", 128), ("wgaT", 128), ("wgbT", 128)]
WB = [("wk2T", 128), ("wv2T", 128), ("wq2T", 128)]
BIAS_NAMES = ["bq1", "bk1", "bq2", "bk2", "bvd", "bvs", "bgh", "bp", "bsh"]


def _r(ap):
    return ap.bitcast(F32R)


def build_program(n_tok=N_TOK, nq=NQ, ch=CH, fd=FD):
    mt = n_tok // 128   # key tiles
    nch = nq // ch      # query chunks per attention
    spc = ch // fd      # matmul slices per chunk

    nc = bacc.Bacc("TRN2", target_bir_lowering=False, debug=False)
    din = {}
    for name in ["x_h", "x_m"]:
        din[name] = nc.dram_tensor(name, [C, n_tok], F16, kind="ExternalInput").ap()
    nwa = sum(w for _, w in WA)
    nwb = sum(w for _, w in WB)
    din["wpackA"] = nc.dram_tensor("wpackA", [C, nwa], F16, kind="ExternalInput").ap()
    din["wpackB"] = nc.dram_tensor("wpackB", [C, nwb], F16, kind="ExternalInput").ap()
    din["wp32"] = nc.dram_tensor("wp32", [C, C], F32, kind="ExternalInput").ap()
    din["bpack"] = nc.dram_tensor(
        "bpack", [C, len(BIAS_NAMES)], F32, kind="ExternalInput"
    ).ap()
    out_d = nc.dram_tensor("out", [C, nq], F32, kind="ExternalOutput").ap()

    with ExitStack() as ctx:
        tc = ctx.enter_context(tile.TileContext(nc))
        const = ctx.enter_context(tc.tile_pool(name="const", bufs=1))
        big = ctx.enter_context(tc.tile_pool(name="big", bufs=1))
        ppool = ctx.enter_context(tc.tile_pool(name="ppool", bufs=5))
        rpool = ctx.enter_context(tc.tile_pool(name="rpool", bufs=2))
        stpool = ctx.enter_context(tc.tile_pool(name="stpool", bufs=2))
        ps_pool = ctx.enter_context(tc.tile_pool(name="ps", bufs=4, space="PSUM"))
        pacc_pool = ctx.enter_context(tc.tile_pool(name="pacc", bufs=2, space="PSUM"))

        # ---- inputs: 1024-col fp16 pieces on both rings, ordered by first
        # use: xm piece 0 + bias pack head the sync ring, weight pack A heads
        # the scalar ring, so the first conv starts as early as possible
        xh_sb = big.tile([C, n_tok], F16, name="xh")
        xm_sb = big.tile([C, n_tok], F16, name="xm")
        wpackA_sb = const.tile([C, nwa], F16, name="wpackA")
        wpackB_sb = const.tile([C, nwb], F16, name="wpackB")
        wp32_sb = const.tile([C, C], F32R, name="wp32")
        bpack_sb = const.tile([C, len(BIAS_NAMES)], F32, name="bpack")
        dch = min(1024, n_tok)
        npc = n_tok // dch
        xs = lambda i: slice(i * dch, (i + 1) * dch)
        # first piece split in half and the K1 weight sent alone: the first
        # conv matmul only needs wk1T + 512 cols of xm, so it dispatches
        # right after ~160KB of descriptors instead of ~500KB
        hp = dch // 2
        nc.scalar.dma_start(out=wpackA_sb[:, :C], in_=din["wpackA"][:, :C])
        nc.sync.dma_start(out=xm_sb[:, 0:hp], in_=din["x_m"][:, 0:hp])
        nc.sync.dma_start(out=xm_sb[:, hp:dch], in_=din["x_m"][:, hp:dch])
        nc.scalar.dma_start(out=wpackA_sb[:, C:], in_=din["wpackA"][:, C:])
        nc.sync.dma_start(out=bpack_sb[:], in_=din["bpack"][:])
        for i in range(1, npc):
            eng = nc.scalar if i % 2 == 1 else nc.sync
            eng.dma_start(out=xm_sb[:, xs(i)], in_=din["x_m"][:, xs(i)])
        for i in range(npc):
            eng = nc.sync if i % 2 == 0 else nc.scalar
            eng.dma_start(out=xh_sb[:, xs(i)], in_=din["x_h"][:, xs(i)])
        nc.sync.dma_start(out=wp32_sb[:], in_=_r(din["wp32"][:]))
        nc.scalar.dma_start(out=wpackB_sb[:], in_=din["wpackB"][:])
        w_sb = {}
        off = 0
        for name, wid in WA:
            w_sb[name] = wpackA_sb[:, off : off + wid]
            off += wid
        off = 0
        for name, wid in WB:
            w_sb[name] = wpackB_sb[:, off : off + wid]
            off += wid
        w_sb["wpTs"] = wp32_sb[:]
        b_sb = {name: bpack_sb[:, i : i + 1] for i, name in enumerate(BIAS_NAMES)}
        ones2 = const.tile([C, 2, C], E4, name="ones2")
        nc.gpsimd.memset(ones2[:], 1.0)

        K1_sb = big.tile([C, n_tok], F32R, name="K1")
        K2_sb = big.tile([C, n_tok], F32R, name="K2")
        Q1_sb = big.tile([C, nq], F32R, name="Q1")
        Q2_sb = big.tile([C, nq], F32R, name="Q2")
        # V^T e4: [tok-in-tile, pair-parity, whichV, pair-major channel col];
        # PV DoubleRow contracts the parity dim = two key tiles per pass
        vpk = big.tile([C, 2, 2, (mt // 2) * 128], E4, name="vpk")
        o1_sb = big.tile([C, nq], F32R, name="o1")
        o2_sb = big.tile([C, nq], F32R, name="o2")
        t_sb = big.tile([C, nq], F32R, name="t")
        tb_sb = big.tile([C, nq], F32R, name="tb")
        d_sb = big.tile([C, nq], F32R, name="d")

        def conv(dst_sb, wT_sb, x_sb, j, bias_sb, ei):
            # per-slice psum half-slots; evictions alternate ACT/DVE so the
            # ring drains at 2x single-engine rate during the conv phase
            for s in range(spc):
                sl = slice(j * ch + s * fd, j * ch + (s + 1) * fd)
                ps = ps_pool.tile([C, fd], F32, tag="ps", name="psc")
                nc.tensor.matmul(ps[:], wT_sb, x_sb[:, sl], start=True, stop=True)
                if (ei + s) % 2 == 0:
                    nc.scalar.activation(
                        dst_sb[:, sl], ps[:],
                        mybir.ActivationFunctionType.Identity, bias=bias_sb,
                    )
                else:
                    nc.vector.tensor_scalar_add(dst_sb[:, sl], ps[:], bias_sb)

        def vt_conv(a, x_sb, wvT_sb, g):
            # V^T (single e4) for key-tile pair 2g, 2g+1 of attention a.
            # fp16 moving operands pay no below-256-col rate penalty, so each
            # key tile is one 128-col matmul.
            ps = ps_pool.tile([C, 2, C], F32, tag="ps", name="psv")
            for u in range(2):
                j = 2 * g + u
                nc.tensor.matmul(
                    ps[:, u, :], x_sb[:, j * 128 : (j + 1) * 128], wvT_sb,
                    start=True, stop=True,
                )
            dst = vpk[:, :, a, g * 128 : (g + 1) * 128]
            if g % 2 == 0:
                nc.scalar.copy(dst, ps[:])
            else:
                nc.vector.tensor_copy(dst, ps[:])

        # ---- conv phase, ordered by input arrival (xm pieces, then xh);
        # vt pairs interleave with K convs: the K matmuls keep the PE busy
        # while vt evictions recycle the other PSUM ring
        ppj = max(1, (ch // 256))  # vt pairs per ch-wide column chunk
        def kv_phase(a, K_dst, wk, wv, x_sb, bk):
            for j in range(n_tok // ch):
                conv(K_dst, wk, x_sb, j, bk, j)
                for u in range(ppj):
                    g = j * ppj + u
                    if g < mt // 2:
                        vt_conv(a, x_sb, wv, g)
            for g in range((n_tok // ch) * ppj, mt // 2):
                vt_conv(a, x_sb, wv, g)

        kv_phase(0, K1_sb, w_sb["wk1T"], w_sb["wv1T"], xm_sb, b_sb["bk1"])
        for j in range(nq // ch):
            conv(Q1_sb, w_sb["wq1T"], xh_sb, j, b_sb["bq1"], j)
        # gate needs only xh/xm cols < nq: compute early, off the critical path
        for j in range(nq // ch):
            for s in range(spc):
                sl = slice(j * ch + s * fd, j * ch + (s + 1) * fd)
                ps = ps_pool.tile([C, fd], F32, tag="ps", name="psg")
                nc.tensor.matmul(ps[:], w_sb["wgaT"], xh_sb[:, sl], start=True, stop=False)
                nc.tensor.matmul(ps[:], w_sb["wgbT"], xm_sb[:, sl], start=False, stop=True)
                nc.scalar.activation(
                    t_sb[:, sl], ps[:],
                    mybir.ActivationFunctionType.Tanh, bias=b_sb["bgh"], scale=0.5,
                )
        kv_phase(1, K2_sb, w_sb["wk2T"], w_sb["wv2T"], xh_sb, b_sb["bk2"])
        for j in range(nq // ch):
            conv(Q2_sb, w_sb["wq2T"], xm_sb, j, b_sb["bq2"], j + 1)
        # u = (1+t')*bv1 + (1-t')*bv2 = t'*(bv1-bv2) + (bv1+bv2): the V biases
        # ride the projection as a third accumulating matmul, so no per-chunk
        # bias adds and no bias wait on the tail chain (ACT, off-path)
        u_sb = big.tile([C, nq], F32R, name="u")
        for j in range(nq // ch):
            sl = slice(j * ch, (j + 1) * ch)
            nc.scalar.activation(
                u_sb[:, sl], t_sb[:, sl],
                mybir.ActivationFunctionType.Identity,
                bias=b_sb["bvs"], scale=b_sb["bvd"],
            )
        # t <- 1+tanh, tb <- 1-tanh (fused = 0.5*[o1*(1+t') + o2*(1-t')], 0.5 in wp)
        for j in range(nq // ch):
            sl = slice(j * ch, (j + 1) * ch)
            nc.gpsimd.tensor_scalar(
                tb_sb[:, sl], t_sb[:, sl], -1.0, 1.0,
                mybir.AluOpType.mult, mybir.AluOpType.add,
            )
            nc.gpsimd.tensor_scalar_add(t_sb[:, sl], t_sb[:, sl], 1.0)

        # Deferred thunks (GpSimd/DVE/PE-proj) drained inside attention loops.
        events = []

        mA = float(SCALE * A8)
        mB = float(B8 - SH * A8)

        def attention(a, o_sb, Q_sb, K_sb, post_chunk=None):
            for cidx in range(nch):
                p_out = pacc_pool.tile([C, ch], F32, tag="acc")
                p_den = pacc_pool.tile([C, ch], F32, tag="acc")
                pts = {}

                def scores(m):
                    # per-slice psum half-slots on a 4-deep ring: the
                    # producer's ~1.0us half-op round trip now fits inside
                    # the ~1.5us half-slot reuse distance, so the PE never
                    # waits on exp
                    ksl = slice(m * 128, (m + 1) * 128)
                    if m % 2 == 0:
                        pts[m // 2] = ppool.tile([C, 2, ch], E4, tag="pt", name="pt")
                    pt_half = pts[m // 2][:, m % 2, :]
                    for s in range(spc):
                        qsl = slice(cidx * ch + s * fd, cidx * ch + (s + 1) * fd)
                        ps = ps_pool.tile([C, fd], F32, tag="ps", name="pss")
                        nc.tensor.matmul(
                            ps[:], K_sb[:, ksl], Q_sb[:, qsl], start=True, stop=True
                        )
                        ssl = slice(s * fd, (s + 1) * fd)
                        if act_tile(m, mt):
                            nc.scalar.activation(
                                pt_half[:, ssl], ps[:],
                                mybir.ActivationFunctionType.Exp,
                                bias=b_sb["bsh"], scale=SCALE,
                            )
                        else:
                            nc.vector.tensor_scalar(
                                pt_half[:, ssl].bitcast(U8), ps[:], mA, mB,
                                mybir.AluOpType.mult, mybir.AluOpType.add,
                            )

                def pv_den(k):
                    # both PV and den contract key-tile pair k in one
                    # DoubleRow pass over the same [pt_2k|pt_2k+1] moving data
                    first, last = k == 0, k == mt // 2 - 1
                    wv = vpk[:, :, a, k * 128 : (k + 1) * 128]
                    for s in range(spc):
                        ssl = slice(s * fd, (s + 1) * fd)
                        nc.tensor.matmul(
                            p_out[:, ssl], wv, pts[k][:, :, ssl],
                            start=first, stop=last,
                            perf_mode=mybir.MatmulPerfMode.DoubleRow,
                        )
                    for s in range(spc):
                        ssl = slice(s * fd, (s + 1) * fd)
                        nc.tensor.matmul(
                            p_den[:, ssl], ones2[:], pts[k][:, :, ssl],
                            start=first, stop=last,
                            perf_mode=mybir.MatmulPerfMode.DoubleRow,
                        )
                    if k >= 2:
                        pts.pop(k - 2)

                # pv_den lags scores by 5 tiles (minimum 3): the first PV
                # of a chunk then fires after the previous chunk's normalize
                # has freed the accumulator slots
                scores(0)
                scores(1)
                for m in range(2, mt):
                    scores(m)
                    if m % 2 == 1 and m >= 7:
                        pv_den((m - 7) // 2)
                    elif m % 2 == 0 and events:
                        events.pop(0)()
                for k in range(max(0, mt // 2 - 3), mt // 2):
                    pv_den(k)

                # reciprocal+normalize in halves: the accumulator half-slots
                # free ~1.4us after the last pv_den instead of ~2.8us; with
                # the lag-7 PV slots this clears the chunk-boundary WAR stall
                rec = rpool.tile([C, ch], F32, tag="rec")
                for s_ in range(spc):
                    ssl = slice(s_ * fd, (s_ + 1) * fd)
                    osl = slice(cidx * ch + s_ * fd, cidx * ch + (s_ + 1) * fd)
                    nc.vector.reciprocal_approx_fast(rec[:, ssl], p_den[:, ssl])
                    nc.vector.tensor_mul(o_sb[:, osl], p_out[:, ssl], rec[:, ssl])

                if post_chunk is not None:
                    events.extend(post_chunk(cidx))

        attention(0, o1_sb, Q1_sb, K1_sb)

        # o1 <- o1*(1+t') runs as soon as attn1 finishes: drains during
        # attn2's early chunks, off the tail critical path
        for c in range(nch):
            for s_ in range(spc):
                sl = slice(c * ch + s_ * fd, c * ch + (s_ + 1) * fd)

                def _ga(sl=sl):
                    nc.vector.tensor_mul(o1_sb[:, sl], o1_sb[:, sl], t_sb[:, sl])

                events.append(_ga)

        def fuse_and_project(cidx):
            # out = wpTs.T @ [o1*(1+t') + o2*(1-t') + u] + bp, with the fuse
            # add and the u bias-term riding the projection's PSUM accumulation
            thunks = []
            for s in range(spc):
                sl = slice(cidx * ch + s * fd, cidx * ch + (s + 1) * fd)

                def _gb(sl=sl):
                    nc.vector.tensor_mul(d_sb[:, sl], o2_sb[:, sl], tb_sb[:, sl])

                def _proj(sl=sl):
                    ps = ps_pool.tile([C, fd], F32, tag="ps", name="psp")
                    nc.tensor.matmul(
                        ps[:], w_sb["wpTs"], o1_sb[:, sl], start=True, stop=False
                    )
                    nc.tensor.matmul(
                        ps[:], w_sb["wpTs"], d_sb[:, sl], start=False, stop=False
                    )
                    nc.tensor.matmul(
                        ps[:], w_sb["wpTs"], u_sb[:, sl], start=False, stop=True
                    )
                    st = stpool.tile([C, fd], F32, tag="st")
                    hq = fd // 2
                    for q_ in range(2):
                        qs = slice(q_ * hq, (q_ + 1) * hq)
                        nc.scalar.activation(
                            st[:, qs], ps[:, qs],
                            mybir.ActivationFunctionType.Identity, bias=b_sb["bp"],
                        )
                        osl = slice(sl.start + q_ * hq, sl.start + (q_ + 1) * hq)
                        nc.sync.dma_start(out=out_d[:, osl], in_=st[:, qs])

                thunks += [_gb, _proj]
            return thunks

        attention(1, o2_sb, Q2_sb, K2_sb, post_chunk=fuse_and_project)
        while events:
            events.pop(0)()

    nc.compile()
    return nc


def make_in_maps(hsi, msi, weights, n_cores=8):
    """Host-side sharding: core i handles (b=i//2, half=i%2); token axis rotated
    so the core's queries are columns [0, NQ)."""
    B = hsi.shape[0]
    hsi = np.ascontiguousarray(hsi.reshape(B, C, N_TOK), dtype=np.float16)
    msi = np.ascontiguousarray(msi.reshape(B, C, N_TOK), dtype=np.float16)
    in_maps = []
    for core in range(n_cores):
        b, h = core // 2, core % 2
        if h == 0:
            x_h, x_m = hsi[b], msi[b]
        else:
            x_h = np.concatenate([hsi[b][:, NQ:], hsi[b][:, :NQ]], axis=1)
            x_m = np.concatenate([msi[b][:, NQ:], msi[b][:, :NQ]], axis=1)
        m = {"x_h": np.ascontiguousarray(x_h), "x_m": np.ascontiguousarray(x_m)}
        m.update(weights)
        in_maps.append(m)
    return in_maps


def make_weight_map(
    wq1, bq1, wk1, bk1, wv1, bv1, wq2, bq2, wk2, bk2, wv2, bv2, wg, bg, wp, bp
):
    f = np.float32
    col = lambda v: np.ascontiguousarray(np.asarray(v, f).reshape(C, 1))
    tr = lambda w: np.ascontiguousarray(np.asarray(w, f).T)
    w = {
        "wq1T": tr(wq1), "wk1T": tr(wk1), "wq2T": tr(wq2), "wk2T": tr(wk2),
        "wgaT": tr(np.asarray(wg, f)[:, :C]),
        "wgbT": tr(np.asarray(wg, f)[:, C:]),
        "wv1T": tr(wv1),
        "wv2T": tr(wv2),
    }
    bv1a = np.asarray(bv1, f)
    bv2a = np.asarray(bv2, f)
    b = {
        "bq1": col(bq1), "bk1": col(bk1), "bq2": col(bq2), "bk2": col(bk2),
        "bvd": col(bv1a - bv2a), "bvs": col(bv1a + bv2a),
        "bgh": col(0.5 * np.asarray(bg, f)),
        "bp": col(bp), "bsh": col(np.full(C, -SH, f)),
    }
    wpackA = np.concatenate([w[n] for n, _ in WA], axis=1)
    wpackB = np.concatenate([w[n] for n, _ in WB], axis=1)
    bpack = np.concatenate([b[n] for n in BIAS_NAMES], axis=1)
    return {
        "wpackA": np.ascontiguousarray(wpackA.astype(np.float16)),
        "wpackB": np.ascontiguousarray(wpackB.astype(np.float16)),
        "wp32": np.ascontiguousarray(tr(0.5 * np.asarray(wp, f))),
        "bpack": np.ascontiguousarray(bpack),
    }


_NC_CACHE = {}


def _get_program():
    if "nc" not in _NC_CACHE:
        _NC_CACHE["nc"] = build_program()
    return _NC_CACHE["nc"]


def run_on_cores(in_maps, trace=False, **kwargs):
    from concourse.bass_utils import run_bass_kernel_spmd

    nc = _get_program()
    return run_bass_kernel_spmd(
        nc, in_maps, core_ids=list(range(len(in_maps))), trace=trace, **kwargs
    )


def kernel(
    hsi, msi, wq1, bq1, wk1, bk1, wv1, bv1, wq2, bq2, wk2, bk2, wv2, bv2,
    wg, bg, wp, bp,
):
    B, _, H, W = hsi.shape
    weights = make_weight_map(
        wq1, bq1, wk1, bk1, wv1, bv1, wq2, bq2, wk2, bk2, wv2, bv2, wg, bg, wp, bp
    )
    in_maps = make_in_maps(np.asarray(hsi), np.asarray(msi), weights)
    res = run_on_cores(in_maps)
    out = np.zeros((B, C, N_TOK), dtype=np.float32)
    for core in range(8):
        b, h = core // 2, core % 2
        out[b][:, h * NQ : (h + 1) * NQ] = res.results[core]["out"]
    return out.reshape(B, C, H, W)


# revision 8
# speedup vs baseline: 1.1972x; 1.0009x over previous
"""CrossAttentionFusion Trainium2 kernel: fp8-e4m3 DoubleRow attention.

Problem (per batch element b of 4, C=128 channels, N=4096 tokens):
    Q1 = wq1@hsi+bq1; K1 = wk1@msi+bk1; V1 = wv1@msi+bv1   (1x1 convs)
    Q2 = wq2@msi+bq2; K2 = wk2@hsi+bk2; V2 = wv2@hsi+bv2
    out1 = attn(Q1,K1,V1); out2 = attn(Q2,K2,V2)           (softmax over keys)
    g = sigmoid(wg@[hsi;msi]+bg)
    out = wp@(g*out1 + (1-g)*out2) + bp

Sharding: 8 cores = (b, query-half); the host rotates the token axis per core
so each core's queries are columns [0, NQ) and the SPMD program is offset-free.

Dataflow (every piece validated in micro_test*.py + exact numpy emulation;
measured end-to-end rel_max 1.45e-2 against the fp32 reference, gate 2e-2):
  - Empirical PE rate law (from traces): ~1 output column/cycle at 2.4GHz for
    every dtype; DoubleRow fp8 doubles CONTRACTION per column, not column
    rate. So wins come from cutting output columns, not switching dtypes.
  - pt = exp(scale*s - SH) stored as fp8-e4m3 (SH=2 keeps exp <= 105 < 240).
    Producers alternate per key tile: ACT tiles use the Exp table with e4
    output; DVE tiles use a uint8-domain Schraudolph (single tensor_scalar;
    the f32->u8 convert rounds-to-nearest and saturates [0,255], and
    bits = A8*(scale*s - SH) + B8 are raw e4m3 bit patterns).
  - PV matmul: fp8 DoubleRow contracts TWO key tiles per pass (weights =
    adjacent V^T tiles, moving = the adjacent [pt_2k|pt_2k+1] pair), halving
    PV output columns vs fp32r. V is single e4m3 (err ~2% RMS, averaged out).
  - softmax denominator: same DoubleRow trick against an e4 ones [128,2,128]
    block, halving den columns and eliminating all DVE pair-add chains.
  - scores stay fp32r (fp8 QK fails the precision gate); x and the conv/gate
    weights ride fp16 (halves input DMA; ~1e-4 noise).
  - PSUM layout: 4-deep ring of [128,512] score half-slots (the exp
    producer's ~1.0us round trip fits inside the ~1.5us half-slot reuse
    distance, so the PE never waits) + 2x2 accumulator slots for PV/den.
  - pv_den lags scores by 5 tiles so a chunk's first PV lands after the
    previous chunk's normalize has freed the accumulator slots.
  - V biases fold into u = (1+t')bv1 + (1-t')bv2 (per-partition affine of
    tanh) and ride the projection PSUM accumulation; the gated fuse-add does
    too, so the tail chain is rec -> normalize -> o2-gate -> project.
  - input DMA: fp16 pieces on both HWDGE rings ordered by first use, with a
    tiny wk1-only weight DMA and a half-size first xm piece so the first conv
    dispatches as early as the descriptor-generation latency allows.
Engine split: ACT = exp tiles + V evictions + tanh + u + proj evictions;
DVE = Schraudolph tiles + K/Q evictions + reciprocal + normalize + fusion
muls; GpSimd (no PSUM access, ~8x slower than modeled on tensor_scalar) =
only the early gate transforms.
"""

import sys

if "/opt/trn_rl_repo" not in sys.path:
    sys.path.insert(0, "/opt/trn_rl_repo")

from contextlib import ExitStack

import numpy as np

import concourse.bacc as bacc
import concourse.bass as bass  # noqa: F401
import concourse.tile as tile
from concourse import mybir

F32 = mybir.dt.float32
F32R = mybir.dt.float32r
F16 = mybir.dt.float16
E4 = mybir.dt.float8e4
U8 = mybir.dt.uint8
C = 128
N_TOK = 4096
NQ = 2048
FD = 512   # matmul moving-operand max for 4-byte dtypes
CH = 1024  # query-chunk width (PSUM accumulator width)
SCALE = 1.0 / float(np.sqrt(np.float32(C)))
SH = 2.0                       # softmax shift: pt = exp(scale*s - SH)
A8 = 8.0 / float(np.log(2.0))  # e4m3 bits per nat
B8 = 56.0 - 0.3434             # e4m3 bias-7 bit offset, RMS-centered
def act_tile(m, mt):
    """pt-producer pick: ACT exp vs DVE Schraudolph. First 3 tiles go to ACT
    (the PE pipeline is still filling, so a serial ACT run is free), then the
    engines strictly alternate — no same-engine run mid-chunk means the
    2-deep PSUM score ring never waits on a busy producer. 18/32 per chunk
    on ACT balances total ACT vs DVE load."""
    return m < 3 or m % 2 == 1

# conv/gate/V weights and x ride in fp16 (~1e-4 relative noise, invisible
# next to the fp8 pt quantization); wpTs stays fp32r (its moving op is f32r).
# Pack A = attention-1 + gate weights (heads its DMA ring), pack B = attn-2.
WA = [("wk1T", 128), ("wv1T", 128), ("wq1T", 128), ("wgaT", 128), ("wgbT", 128)]
WB = [("wk2T", 128), ("wv2T", 128), ("wq2T", 128)]
BIAS_NAMES = ["bq1", "bk1", "bq2", "bk2", "bvd", "bvs", "bgh", "bp", "bsh"]


def _r(ap):
    return ap.bitcast(F32R)


def build_program(n_tok=N_TOK, nq=NQ, ch=CH, fd=FD):
    mt = n_tok // 128   # key tiles
    nch = nq // ch      # query chunks per attention
    spc = ch // fd      # matmul slices per chunk

    nc = bacc.Bacc("TRN2", target_bir_lowering=False, debug=False)
    din = {}
    for name in ["x_h", "x_m"]:
        din[name] = nc.dram_tensor(name, [C, n_tok], F16, kind="ExternalInput").ap()
    nwa = sum(w for _, w in WA)
    nwb = sum(w for _, w in WB)
    din["wpackA"] = nc.dram_tensor("wpackA", [C, nwa], F16, kind="ExternalInput").ap()
    din["wpackB"] = nc.dram_tensor("wpackB", [C, nwb], F16, kind="ExternalInput").ap()
    din["wp32"] = nc.dram_tensor("wp32", [C, C], F32, kind="ExternalInput").ap()
    din["bpack"] = nc.dram_tensor(
        "bpack", [C, len(BIAS_NAMES)], F32, kind="ExternalInput"
    ).ap()
    out_d = nc.dram_tensor("out", [C, nq], F32, kind="ExternalOutput").ap()

    with ExitStack() as ctx:
        tc = ctx.enter_context(tile.TileContext(nc))
        const = ctx.enter_context(tc.tile_pool(name="const", bufs=1))
        big = ctx.enter_context(tc.tile_pool(name="big", bufs=1))
        ppool = ctx.enter_context(tc.tile_pool(name="ppool", bufs=4))
        rpool = ctx.enter_context(tc.tile_pool(name="rpool", bufs=2))
        stpool = ctx.enter_context(tc.tile_pool(name="stpool", bufs=2))
        ps_pool = ctx.enter_context(tc.tile_pool(name="ps", bufs=4, space="PSUM"))
        pacc_pool = ctx.enter_context(tc.tile_pool(name="pacc", bufs=2, space="PSUM"))

        # ---- inputs: 1024-col fp16 pieces on both rings, ordered by first
        # use: xm piece 0 + bias pack head the sync ring, weight pack A heads
        # the scalar ring, so the first conv starts as early as possible
        xh_sb = big.tile([C, n_tok], F16, name="xh")
        xm_sb = big.tile([C, n_tok], F16, name="xm")
        wpackA_sb = const.tile([C, nwa], F16, name="wpackA")
        wpackB_sb = const.tile([C, nwb], F16, name="wpackB")
        wp32_sb = const.tile([C, C], F32R, name="wp32")
        bpack_sb = const.tile([C, len(BIAS_NAMES)], F32, name="bpack")
        dch = min(1024, n_tok)
        npc = n_tok // dch
        xs = lambda i: slice(i * dch, (i + 1) * dch)
        # first piece split in half and the K1 weight sent alone: the first
        # conv matmul only needs wk1T + 512 cols of xm, so it dispatches
        # right after ~160KB of descriptors instead of ~500KB
        hp = dch // 2
        nc.scalar.dma_start(out=wpackA_sb[:, :C], in_=din["wpackA"][:, :C])
        nc.sync.dma_start(out=xm_sb[:, 0:hp], in_=din["x_m"][:, 0:hp])
        nc.sync.dma_start(out=xm_sb[:, hp:dch], in_=din["x_m"][:, hp:dch])
        nc.scalar.dma_start(out=wpackA_sb[:, C:], in_=din["wpackA"][:, C:])
        nc.sync.dma_start(out=bpack_sb[:], in_=din["bpack"][:])
        for i in range(1, npc):
            eng = nc.scalar if i % 2 == 1 else nc.sync
            eng.dma_start(out=xm_sb[:, xs(i)], in_=din["x_m"][:, xs(i)])
        for i in range(npc):
            eng = nc.sync if i % 2 == 0 else nc.scalar
            eng.dma_start(out=xh_sb[:, xs(i)], in_=din["x_h"][:, xs(i)])
        nc.sync.dma_start(out=wp32_sb[:], in_=_r(din["wp32"][:]))
        nc.scalar.dma_start(out=wpackB_sb[:], in_=din["wpackB"][:])
        w_sb = {}
        off = 0
        for name, wid in WA:
            w_sb[name] = wpackA_sb[:, off : off + wid]
            off += wid
        off = 0
        for name, wid in WB:
            w_sb[name] = wpackB_sb[:, off : off + wid]
            off += wid
        w_sb["wpTs"] = wp32_sb[:]
        b_sb = {name: bpack_sb[:, i : i + 1] for i, name in enumerate(BIAS_NAMES)}
        ones2 = const.tile([C, 2, C], E4, name="ones2")
        nc.gpsimd.memset(ones2[:], 1.0)

        K1_sb = big.tile([C, n_tok], F32R, name="K1")
        K2_sb = big.tile([C, n_tok], F32R, name="K2")
        Q1_sb = big.tile([C, nq], F32R, name="Q1")
        Q2_sb = big.tile([C, nq], F32R, name="Q2")
        # V^T e4: [tok-in-tile, pair-parity, whichV, pair-major channel col];
        # PV DoubleRow contracts the parity dim = two key tiles per pass
        vpk = big.tile([C, 2, 2, (mt // 2) * 128], E4, name="vpk")
        o1_sb = big.tile([C, nq], F32R, name="o1")
        o2_sb = big.tile([C, nq], F32R, name="o2")
        t_sb = big.tile([C, nq], F32R, name="t")
        tb_sb = big.tile([C, nq], F32R, name="tb")
        d_sb = big.tile([C, nq], F32R, name="d")

        def conv(dst_sb, wT_sb, x_sb, j, bias_sb, ei):
            # per-slice psum half-slots; evictions alternate ACT/DVE so the
            # ring drains at 2x single-engine rate during the conv phase
            for s in range(spc):
                sl = slice(j * ch + s * fd, j * ch + (s + 1) * fd)
                ps = ps_pool.tile([C, fd], F32, tag="ps", name="psc")
                nc.tensor.matmul(ps[:], wT_sb, x_sb[:, sl], start=True, stop=True)
                if (ei + s) % 2 == 0:
                    nc.scalar.activation(
                        dst_sb[:, sl], ps[:],
                        mybir.ActivationFunctionType.Identity, bias=bias_sb,
                    )
                else:
                    nc.vector.tensor_scalar_add(dst_sb[:, sl], ps[:], bias_sb)

        def vt_conv(a, x_sb, wvT_sb, g):
            # V^T (single e4) for key-tile pair 2g, 2g+1 of attention a.
            # fp16 moving operands pay no below-256-col rate penalty, so each
            # key tile is one 128-col matmul.
            ps = ps_pool.tile([C, 2, C], F32, tag="ps", name="psv")
            for u in range(2):
                j = 2 * g + u
                nc.tensor.matmul(
                    ps[:, u, :], x_sb[:, j * 128 : (j + 1) * 128], wvT_sb,
                    start=True, stop=True,
                )
            dst = vpk[:, :, a, g * 128 : (g + 1) * 128]
            if g % 2 == 0:
                nc.scalar.copy(dst, ps[:])
            else:
                nc.vector.tensor_copy(dst, ps[:])

        # ---- conv phase, ordered by input arrival (xm pieces, then xh);
        # vt pairs interleave with K convs: the K matmuls keep the PE busy
        # while vt evictions recycle the other PSUM ring
        ppj = max(1, (ch // 256))  # vt pairs per ch-wide column chunk
        def kv_phase(a, K_dst, wk, wv, x_sb, bk):
            for j in range(n_tok // ch):
                conv(K_dst, wk, x_sb, j, bk, j)
                for u in range(ppj):
                    g = j * ppj + u
                    if g < mt // 2:
                        vt_conv(a, x_sb, wv, g)
            for g in range((n_tok // ch) * ppj, mt // 2):
                vt_conv(a, x_sb, wv, g)

        kv_phase(0, K1_sb, w_sb["wk1T"], w_sb["wv1T"], xm_sb, b_sb["bk1"])
        for j in range(nq // ch):
            conv(Q1_sb, w_sb["wq1T"], xh_sb, j, b_sb["bq1"], j)
        # gate needs only xh/xm cols < nq: compute early, off the critical path
        for j in range(nq // ch):
            for s in range(spc):
                sl = slice(j * ch + s * fd, j * ch + (s + 1) * fd)
                ps = ps_pool.tile([C, fd], F32, tag="ps", name="psg")
                nc.tensor.matmul(ps[:], w_sb["wgaT"], xh_sb[:, sl], start=True, stop=False)
                nc.tensor.matmul(ps[:], w_sb["wgbT"], xm_sb[:, sl], start=False, stop=True)
                nc.scalar.activation(
                    t_sb[:, sl], ps[:],
                    mybir.ActivationFunctionType.Tanh, bias=b_sb["bgh"], scale=0.5,
                )
        kv_phase(1, K2_sb, w_sb["wk2T"], w_sb["wv2T"], xh_sb, b_sb["bk2"])
        for j in range(nq // ch):
            conv(Q2_sb, w_sb["wq2T"], xm_sb, j, b_sb["bq2"], j + 1)
        # u = (1+t')*bv1 + (1-t')*bv2 = t'*(bv1-bv2) + (bv1+bv2): the V biases
        # ride the projection as a third accumulating matmul, so no per-chunk
        # bias adds and no bias wait on the tail chain (ACT, off-path)
        u_sb = big.tile([C, nq], F32R, name="u")
        for j in range(nq // ch):
            sl = slice(j * ch, (j + 1) * ch)
            nc.scalar.activation(
                u_sb[:, sl], t_sb[:, sl],
                mybir.ActivationFunctionType.Identity,
                bias=b_sb["bvs"], scale=b_sb["bvd"],
            )
        # t <- 1+tanh, tb <- 1-tanh (fused = 0.5*[o1*(1+t') + o2*(1-t')], 0.5 in wp)
        for j in range(nq // ch):
            sl = slice(j * ch, (j + 1) * ch)
            nc.gpsimd.tensor_scalar(
                tb_sb[:, sl], t_sb[:, sl], -1.0, 1.0,
                mybir.AluOpType.mult, mybir.AluOpType.add,
            )
            nc.gpsimd.tensor_scalar_add(t_sb[:, sl], t_sb[:, sl], 1.0)

        # Deferred thunks (GpSimd/DVE/PE-proj) drained inside attention loops.
        events = []

        mA = float(SCALE * A8)
        mB = float(B8 - SH * A8)

        def attention(a, o_sb, Q_sb, K_sb, post_chunk=None):
            for cidx in range(nch):
                p_out = pacc_pool.tile([C, ch], F32, tag="acc")
                p_den = pacc_pool.tile([C, ch], F32, tag="acc")
                pts = {}

                def scores(m):
                    # per-slice psum half-slots on a 4-deep ring: the
                    # producer's ~1.0us half-op round trip now fits inside
                    # the ~1.5us half-slot reuse distance, so the PE never
                    # waits on exp
                    ksl = slice(m * 128, (m + 1) * 128)
                    if m % 2 == 0:
                        pts[m // 2] = ppool.tile([C, 2, ch], E4, tag="pt", name="pt")
                    pt_half = pts[m // 2][:, m % 2, :]
                    for s in range(spc):
                        qsl = slice(cidx * ch + s * fd, cidx * ch + (s + 1) * fd)
                        ps = ps_pool.tile([C, fd], F32, tag="ps", name="pss")
                        nc.tensor.matmul(
                            ps[:], K_sb[:, ksl], Q_sb[:, qsl], start=True, stop=True
                        )
                        ssl = slice(s * fd, (s + 1) * fd)
                        if act_tile(m, mt):
                            nc.scalar.activation(
                                pt_half[:, ssl], ps[:],
                                mybir.ActivationFunctionType.Exp,
                                bias=b_sb["bsh"], scale=SCALE,
                            )
                        else:
                            nc.vector.tensor_scalar(
                                pt_half[:, ssl].bitcast(U8), ps[:], mA, mB,
                                mybir.AluOpType.mult, mybir.AluOpType.add,
                            )

                def pv_den(k):
                    # both PV and den contract key-tile pair k in one
                    # DoubleRow pass over the same [pt_2k|pt_2k+1] moving data
                    first, last = k == 0, k == mt // 2 - 1
                    wv = vpk[:, :, a, k * 128 : (k + 1) * 128]
                    for s in range(spc):
                        ssl = slice(s * fd, (s + 1) * fd)
                        nc.tensor.matmul(
                            p_out[:, ssl], wv, pts[k][:, :, ssl],
                            start=first, stop=last,
                            perf_mode=mybir.MatmulPerfMode.DoubleRow,
                        )
                    for s in range(spc):
                        ssl = slice(s * fd, (s + 1) * fd)
                        nc.tensor.matmul(
                            p_den[:, ssl], ones2[:], pts[k][:, :, ssl],
                            start=first, stop=last,
                            perf_mode=mybir.MatmulPerfMode.DoubleRow,
                        )
                    if k >= 2:
                        pts.pop(k - 2)

                # pv_den lags scores by 5 tiles (minimum 3): the first PV
                # of a chunk then fires after the previous chunk's normalize
                # has freed the accumulator slots
                scores(0)
                scores(1)
                for m in range(2, mt):
                    scores(m)
                    if m % 2 == 1 and m >= 5:
                        pv_den((m - 5) // 2)
                    elif m % 2 == 0 and events:
                        events.pop(0)()
                for k in (mt // 2 - 2, mt // 2 - 1):
                    pv_den(k)

                rec = rpool.tile([C, ch], F32, tag="rec")
                if a == 1 and cidx == nch - 1:
                    # tail chunk: halves let the fuse+projection chain start
                    # ~0.7us earlier
                    for s_ in range(spc):
                        ssl = slice(s_ * fd, (s_ + 1) * fd)
                        osl = slice(cidx * ch + s_ * fd, cidx * ch + (s_ + 1) * fd)
                        nc.vector.reciprocal_approx_fast(rec[:, ssl], p_den[:, ssl])
                        nc.vector.tensor_mul(o_sb[:, osl], p_out[:, ssl], rec[:, ssl])
                else:
                    nc.vector.reciprocal_approx_fast(rec[:], p_den[:])
                    osl = slice(cidx * ch, (cidx + 1) * ch)
                    nc.vector.tensor_mul(o_sb[:, osl], p_out[:], rec[:])

                if post_chunk is not None:
                    events.extend(post_chunk(cidx))

        attention(0, o1_sb, Q1_sb, K1_sb)

        # o1 <- o1*(1+t') runs as soon as attn1 finishes: drains during
        # attn2's early chunks, off the tail critical path
        for c in range(nch):
            for s_ in range(spc):
                sl = slice(c * ch + s_ * fd, c * ch + (s_ + 1) * fd)

                def _ga(sl=sl):
                    nc.vector.tensor_mul(o1_sb[:, sl], o1_sb[:, sl], t_sb[:, sl])

                events.append(_ga)

        def fuse_and_project(cidx):
            # out = wpTs.T @ [o1*(1+t') + o2*(1-t') + u] + bp, with the fuse
            # add and the u bias-term riding the projection's PSUM accumulation
            thunks = []
            for s in range(spc):
                sl = slice(cidx * ch + s * fd, cidx * ch + (s + 1) * fd)

                def _gb(sl=sl):
                    nc.vector.tensor_mul(d_sb[:, sl], o2_sb[:, sl], tb_sb[:, sl])

                def _proj(sl=sl):
                    ps = ps_pool.tile([C, fd], F32, tag="ps", name="psp")
                    nc.tensor.matmul(
                        ps[:], w_sb["wpTs"], o1_sb[:, sl], start=True, stop=False
                    )
                    nc.tensor.matmul(
                        ps[:], w_sb["wpTs"], d_sb[:, sl], start=False, stop=False
                    )
                    nc.tensor.matmul(
                        ps[:], w_sb["wpTs"], u_sb[:, sl], start=False, stop=True
                    )
                    st = stpool.tile([C, fd], F32, tag="st")
                    hq = fd // 2
                    for q_ in range(2):
                        qs = slice(q_ * hq, (q_ + 1) * hq)
                        nc.scalar.activation(
                            st[:, qs], ps[:, qs],
                            mybir.ActivationFunctionType.Identity, bias=b_sb["bp"],
                        )
                        osl = slice(sl.start + q_ * hq, sl.start + (q_ + 1) * hq)
                        nc.sync.dma_start(out=out_d[:, osl], in_=st[:, qs])

                thunks += [_gb, _proj]
            return thunks

        attention(1, o2_sb, Q2_sb, K2_sb, post_chunk=fuse_and_project)
        while events:
            events.pop(0)()

    nc.compile()
    return nc


def make_in_maps(hsi, msi, weights, n_cores=8):
    """Host-side sharding: core i handles (b=i//2, half=i%2); token axis rotated
    so the core's queries are columns [0, NQ)."""
    B = hsi.shape[0]
    hsi = np.ascontiguousarray(hsi.reshape(B, C, N_TOK), dtype=np.float16)
    msi = np.ascontiguousarray(msi.reshape(B, C, N_TOK), dtype=np.float16)
    in_maps = []
    for core in range(n_cores):
        b, h = core // 2, core % 2
        if h == 0:
            x_h, x_m = hsi[b], msi[b]
        else:
            x_h = np.concatenate([hsi[b][:, NQ:], hsi[b][:, :NQ]], axis=1)
            x_m = np.concatenate([msi[b][:, NQ:], msi[b][:, :NQ]], axis=1)
        m = {"x_h": np.ascontiguousarray(x_h), "x_m": np.ascontiguousarray(x_m)}
        m.update(weights)
        in_maps.append(m)
    return in_maps


def make_weight_map(
    wq1, bq1, wk1, bk1, wv1, bv1, wq2, bq2, wk2, bk2, wv2, bv2, wg, bg, wp, bp
):
    f = np.float32
    col = lambda v: np.ascontiguousarray(np.asarray(v, f).reshape(C, 1))
    tr = lambda w: np.ascontiguousarray(np.asarray(w, f).T)
    w = {
        "wq1T": tr(wq1), "wk1T": tr(wk1), "wq2T": tr(wq2), "wk2T": tr(wk2),
        "wgaT": tr(np.asarray(wg, f)[:, :C]),
        "wgbT": tr(np.asarray(wg, f)[:, C:]),
        "wv1T": tr(wv1),
        "wv2T": tr(wv2),
    }
    bv1a = np.asarray(bv1, f)
    bv2a = np.asarray(bv2, f)
    b = {
        "bq1": col(bq1), "bk1": col(bk1), "bq2": col(bq2), "bk2": col(bk2),
        "bvd": col(bv1a - bv2a), "bvs": col(bv1a + bv2a),
        "bgh": col(0.5 * np.asarray(bg, f)),
        "bp": col(bp), "bsh": col(np.full(C, -SH, f)),
    }
    wpackA = np.concatenate([w[n] for n, _ in WA], axis=1)
    wpackB = np.concatenate([w[n] for n, _ in WB], axis=1)
    bpack = np.concatenate([b[n] for n in BIAS_NAMES], axis=1)
    return {
        "wpackA": np.ascontiguousarray(wpackA.astype(np.float16)),
        "wpackB": np.ascontiguousarray(wpackB.astype(np.float16)),
        "wp32": np.ascontiguousarray(tr(0.5 * np.asarray(wp, f))),
        "bpack": np.ascontiguousarray(bpack),
    }


_NC_CACHE = {}


def _get_program():
    if "nc" not in _NC_CACHE:
        _NC_CACHE["nc"] = build_program()
    return _NC_CACHE["nc"]


def run_on_cores(in_maps, trace=False, **kwargs):
    from concourse.bass_utils import run_bass_kernel_spmd

    nc = _get_program()
    return run_bass_kernel_spmd(
        nc, in_maps, core_ids=list(range(len(in_maps))), trace=trace, **kwargs
    )


def kernel(
    hsi, msi, wq1, bq1, wk1, bk1, wv1, bv1, wq2, bq2, wk2, bk2, wv2, bv2,
    wg, bg, wp, bp,
):
    B, _, H, W = hsi.shape
    weights = make_weight_map(
        wq1, bq1, wk1, bk1, wv1, bv1, wq2, bq2, wk2, bk2, wv2, bv2, wg, bg, wp, bp
    )
    in_maps = make_in_maps(np.asarray(hsi), np.asarray(msi), weights)
    res = run_on_cores(in_maps)
    out = np.zeros((B, C, N_TOK), dtype=np.float32)
    for core in range(8):
        b, h = core // 2, core % 2
        out[b][:, h * NQ : (h + 1) * NQ] = res.results[core]["out"]
    return out.reshape(B, C, H, W)


# revision 10
# speedup vs baseline: 1.2034x; 1.0051x over previous
"""CrossAttentionFusion Trainium2 kernel: fp8-e4m3 DoubleRow attention.

Problem (per batch element b of 4, C=128 channels, N=4096 tokens):
    Q1 = wq1@hsi+bq1; K1 = wk1@msi+bk1; V1 = wv1@msi+bv1   (1x1 convs)
    Q2 = wq2@msi+bq2; K2 = wk2@hsi+bk2; V2 = wv2@hsi+bv2
    out1 = attn(Q1,K1,V1); out2 = attn(Q2,K2,V2)           (softmax over keys)
    g = sigmoid(wg@[hsi;msi]+bg)
    out = wp@(g*out1 + (1-g)*out2) + bp

Sharding: 8 cores = (b, query-half); the host rotates the token axis per core
so each core's queries are columns [0, NQ) and the SPMD program is offset-free.

Dataflow (every piece validated in micro_test*.py + exact numpy emulation;
measured end-to-end rel_max 1.45e-2 against the fp32 reference, gate 2e-2):
  - Empirical PE rate law (from traces): ~1 output column/cycle at 2.4GHz for
    every dtype; DoubleRow fp8 doubles CONTRACTION per column, not column
    rate. So wins come from cutting output columns, not switching dtypes.
  - pt = exp(scale*s - SH) stored as fp8-e4m3 (SH=2 keeps exp <= 105 < 240).
    Producers alternate per key tile: ACT tiles use the Exp table with e4
    output; DVE tiles use a uint8-domain Schraudolph (single tensor_scalar;
    the f32->u8 convert rounds-to-nearest and saturates [0,255], and
    bits = A8*(scale*s - SH) + B8 are raw e4m3 bit patterns).
  - PV matmul: fp8 DoubleRow contracts TWO key tiles per pass (weights =
    adjacent V^T tiles, moving = the adjacent [pt_2k|pt_2k+1] pair), halving
    PV output columns vs fp32r. V is single e4m3 (err ~2% RMS, averaged out).
  - softmax denominator: same DoubleRow trick against an e4 ones [128,2,128]
    block, halving den columns and eliminating all DVE pair-add chains.
  - scores stay fp32r (fp8 QK fails the precision gate); x and the conv/gate
    weights ride fp16 (halves input DMA; ~1e-4 noise).
  - PSUM layout: 4-deep ring of [128,512] score half-slots (the exp
    producer's ~1.0us round trip fits inside the ~1.5us half-slot reuse
    distance, so the PE never waits) + 2x2 accumulator slots for PV/den.
  - pv_den lags scores by 5 tiles so a chunk's first PV lands after the
    previous chunk's normalize has freed the accumulator slots.
  - V biases fold into u = (1+t')bv1 + (1-t')bv2 (per-partition affine of
    tanh) and ride the projection PSUM accumulation; the gated fuse-add does
    too, so the tail chain is rec -> normalize -> o2-gate -> project.
  - input DMA: fp16 pieces on both HWDGE rings ordered by first use, with a
    tiny wk1-only weight DMA and a half-size first xm piece so the first conv
    dispatches as early as the descriptor-generation latency allows.
Engine split: ACT = exp tiles + V evictions + tanh + u + proj evictions;
DVE = Schraudolph tiles + K/Q evictions + reciprocal + normalize + fusion
muls; GpSimd (no PSUM access, ~8x slower than modeled on tensor_scalar) =
only the early gate transforms.
"""

import sys

if "/opt/trn_rl_repo" not in sys.path:
    sys.path.insert(0, "/opt/trn_rl_repo")

from contextlib import ExitStack

import numpy as np

import concourse.bacc as bacc
import concourse.bass as bass  # noqa: F401
import concourse.tile as tile
from concourse import mybir

F32 = mybir.dt.float32
F32R = mybir.dt.float32r
F16 = mybir.dt.float16
E4 = mybir.dt.float8e4
U8 = mybir.dt.uint8
C = 128
N_TOK = 4096
NQ = 2048
FD = 512   # matmul moving-operand max for 4-byte dtypes
CH = 1024  # query-chunk width (PSUM accumulator width)
SCALE = 1.0 / float(np.sqrt(np.float32(C)))
SH = 2.0                       # softmax shift: pt = exp(scale*s - SH)
A8 = 8.0 / float(np.log(2.0))  # e4m3 bits per nat
B8 = 56.0 - 0.3434             # e4m3 bias-7 bit offset, RMS-centered
def act_tile(m, mt):
    """pt-producer pick: ACT exp vs DVE Schraudolph. First 3 tiles go to ACT
    (the PE pipeline is still filling, so a serial ACT run is free), then the
    engines strictly alternate — no same-engine run mid-chunk means the
    2-deep PSUM score ring never waits on a busy producer. 18/32 per chunk
    on ACT balances total ACT vs DVE load."""
    return m < 3 or m % 2 == 1

# conv/gate/V weights and x ride in fp16 (~1e-4 relative noise, invisible
# next to the fp8 pt quantization); wpTs stays fp32r (its moving op is f32r).
# Pack A = attention-1 + gate weights (heads its DMA ring), pack B = attn-2.
WA = [("wk1T", 128), ("wv1T", 128), ("wq1T", 128), ("wgaT", 128), ("wgbT", 128)]
WB = [("wk2T", 128), ("wv2T", 128), ("wq2T", 128)]
BIAS_NAMES = ["bq1", "bk1", "bq2", "bk2", "bvd", "bvs", "bgh", "bp", "bsh"]


def _r(ap):
    return ap.bitcast(F32R)


def build_program(n_tok=N_TOK, nq=NQ, ch=CH, fd=FD):
    mt = n_tok // 128   # key tiles
    nch = nq // ch      # query chunks per attention
    spc = ch // fd      # matmul slices per chunk

    nc = bacc.Bacc("TRN2", target_bir_lowering=False, debug=False)
    din = {}
    for name in ["x_h", "x_m"]:
        din[name] = nc.dram_tensor(name, [C, n_tok], F16, kind="ExternalInput").ap()
    nwa = sum(w for _, w in WA)
    nwb = sum(w for _, w in WB)
    din["wpackA"] = nc.dram_tensor("wpackA", [C, nwa], F16, kind="ExternalInput").ap()
    din["wpackB"] = nc.dram_tensor("wpackB", [C, nwb], F16, kind="ExternalInput").ap()
    din["wp32"] = nc.dram_tensor("wp32", [C, C], F32, kind="ExternalInput").ap()
    din["bpack"] = nc.dram_tensor(
        "bpack", [C, len(BIAS_NAMES)], F32, kind="ExternalInput"
    ).ap()
    out_d = nc.dram_tensor("out", [C, nq], F32, kind="ExternalOutput").ap()

    with ExitStack() as ctx:
        tc = ctx.enter_context(tile.TileContext(nc))
        const = ctx.enter_context(tc.tile_pool(name="const", bufs=1))
        big = ctx.enter_context(tc.tile_pool(name="big", bufs=1))
        ppool = ctx.enter_context(tc.tile_pool(name="ppool", bufs=4))
        rpool = ctx.enter_context(tc.tile_pool(name="rpool", bufs=2))
        stpool = ctx.enter_context(tc.tile_pool(name="stpool", bufs=2))
        ps_pool = ctx.enter_context(tc.tile_pool(name="ps", bufs=4, space="PSUM"))
        pacc_pool = ctx.enter_context(tc.tile_pool(name="pacc", bufs=2, space="PSUM"))

        # ---- inputs: 1024-col fp16 pieces on both rings, ordered by first
        # use: xm piece 0 + bias pack head the sync ring, weight pack A heads
        # the scalar ring, so the first conv starts as early as possible
        xh_sb = big.tile([C, n_tok], F16, name="xh")
        xm_sb = big.tile([C, n_tok], F16, name="xm")
        wpackA_sb = const.tile([C, nwa], F16, name="wpackA")
        wpackB_sb = const.tile([C, nwb], F16, name="wpackB")
        wp32_sb = const.tile([C, C], F32R, name="wp32")
        bpack_sb = const.tile([C, len(BIAS_NAMES)], F32, name="bpack")
        dch = min(1024, n_tok)
        npc = n_tok // dch
        xs = lambda i: slice(i * dch, (i + 1) * dch)
        # first piece split in half and the K1 weight sent alone: the first
        # conv matmul only needs wk1T + 512 cols of xm, so it dispatches
        # right after ~160KB of descriptors instead of ~500KB
        hp = dch // 2
        nc.scalar.dma_start(out=wpackA_sb[:, :C], in_=din["wpackA"][:, :C])
        nc.sync.dma_start(out=xm_sb[:, 0:hp], in_=din["x_m"][:, 0:hp])
        nc.sync.dma_start(out=xm_sb[:, hp:dch], in_=din["x_m"][:, hp:dch])
        nc.scalar.dma_start(out=wpackA_sb[:, C:], in_=din["wpackA"][:, C:])
        nc.sync.dma_start(out=bpack_sb[:], in_=din["bpack"][:])
        for i in range(1, npc):
            eng = nc.scalar if i % 2 == 1 else nc.sync
            eng.dma_start(out=xm_sb[:, xs(i)], in_=din["x_m"][:, xs(i)])
        for i in range(npc):
            eng = nc.sync if i % 2 == 0 else nc.scalar
            eng.dma_start(out=xh_sb[:, xs(i)], in_=din["x_h"][:, xs(i)])
        nc.sync.dma_start(out=wp32_sb[:], in_=_r(din["wp32"][:]))
        nc.scalar.dma_start(out=wpackB_sb[:], in_=din["wpackB"][:])
        w_sb = {}
        off = 0
        for name, wid in WA:
            w_sb[name] = wpackA_sb[:, off : off + wid]
            off += wid
        off = 0
        for name, wid in WB:
            w_sb[name] = wpackB_sb[:, off : off + wid]
            off += wid
        w_sb["wpTs"] = wp32_sb[:]
        b_sb = {name: bpack_sb[:, i : i + 1] for i, name in enumerate(BIAS_NAMES)}
        ones2 = const.tile([C, 2, C], E4, name="ones2")
        nc.gpsimd.memset(ones2[:], 1.0)

        K1_sb = big.tile([C, n_tok], F32R, name="K1")
        K2_sb = big.tile([C, n_tok], F32R, name="K2")
        Q1_sb = big.tile([C, nq], F32R, name="Q1")
        Q2_sb = big.tile([C, nq], F32R, name="Q2")
        # V^T e4: [tok-in-tile, pair-parity, whichV, pair-major channel col];
        # PV DoubleRow contracts the parity dim = two key tiles per pass
        vpk = big.tile([C, 2, 2, (mt // 2) * 128], E4, name="vpk")
        o1_sb = big.tile([C, nq], F32R, name="o1")
        o2_sb = big.tile([C, nq], F32R, name="o2")
        t_sb = big.tile([C, nq], F32R, name="t")
        tb_sb = big.tile([C, nq], F32R, name="tb")
        d_sb = big.tile([C, nq], F32R, name="d")

        def conv(dst_sb, wT_sb, x_sb, j, bias_sb, ei):
            # per-slice psum half-slots; evictions alternate ACT/DVE so the
            # ring drains at 2x single-engine rate during the conv phase
            for s in range(spc):
                sl = slice(j * ch + s * fd, j * ch + (s + 1) * fd)
                ps = ps_pool.tile([C, fd], F32, tag="ps", name="psc")
                nc.tensor.matmul(ps[:], wT_sb, x_sb[:, sl], start=True, stop=True)
                if (ei + s) % 2 == 0:
                    nc.scalar.activation(
                        dst_sb[:, sl], ps[:],
                        mybir.ActivationFunctionType.Identity, bias=bias_sb,
                    )
                else:
                    nc.vector.tensor_scalar_add(dst_sb[:, sl], ps[:], bias_sb)

        def vt_conv(a, x_sb, wvT_sb, g):
            # V^T (single e4) for key-tile pair 2g, 2g+1 of attention a.
            # fp16 moving operands pay no below-256-col rate penalty, so each
            # key tile is one 128-col matmul.
            ps = ps_pool.tile([C, 2, C], F32, tag="ps", name="psv")
            for u in range(2):
                j = 2 * g + u
                nc.tensor.matmul(
                    ps[:, u, :], x_sb[:, j * 128 : (j + 1) * 128], wvT_sb,
                    start=True, stop=True,
                )
            dst = vpk[:, :, a, g * 128 : (g + 1) * 128]
            if g % 2 == 0:
                nc.scalar.copy(dst, ps[:])
            else:
                nc.vector.tensor_copy(dst, ps[:])

        # ---- conv phase, ordered by input arrival (xm pieces, then xh);
        # vt pairs interleave with K convs: the K matmuls keep the PE busy
        # while vt evictions recycle the other PSUM ring
        ppj = max(1, (ch // 256))  # vt pairs per ch-wide column chunk
        def kv_phase(a, K_dst, wk, wv, x_sb, bk):
            for j in range(n_tok // ch):
                conv(K_dst, wk, x_sb, j, bk, j)
                for u in range(ppj):
                    g = j * ppj + u
                    if g < mt // 2:
                        vt_conv(a, x_sb, wv, g)
            for g in range((n_tok // ch) * ppj, mt // 2):
                vt_conv(a, x_sb, wv, g)

        kv_phase(0, K1_sb, w_sb["wk1T"], w_sb["wv1T"], xm_sb, b_sb["bk1"])
        for j in range(nq // ch):
            conv(Q1_sb, w_sb["wq1T"], xh_sb, j, b_sb["bq1"], j)
        # gate needs only xh/xm cols < nq: compute early, off the critical path
        for j in range(nq // ch):
            for s in range(spc):
                sl = slice(j * ch + s * fd, j * ch + (s + 1) * fd)
                ps = ps_pool.tile([C, fd], F32, tag="ps", name="psg")
                nc.tensor.matmul(ps[:], w_sb["wgaT"], xh_sb[:, sl], start=True, stop=False)
                nc.tensor.matmul(ps[:], w_sb["wgbT"], xm_sb[:, sl], start=False, stop=True)
                nc.scalar.activation(
                    t_sb[:, sl], ps[:],
                    mybir.ActivationFunctionType.Tanh, bias=b_sb["bgh"], scale=0.5,
                )
        kv_phase(1, K2_sb, w_sb["wk2T"], w_sb["wv2T"], xh_sb, b_sb["bk2"])
        for j in range(nq // ch):
            conv(Q2_sb, w_sb["wq2T"], xm_sb, j, b_sb["bq2"], j + 1)
        # u = (1+t')*bv1 + (1-t')*bv2 = t'*(bv1-bv2) + (bv1+bv2): the V biases
        # ride the projection as a third accumulating matmul, so no per-chunk
        # bias adds and no bias wait on the tail chain (ACT, off-path)
        u_sb = big.tile([C, nq], F32R, name="u")
        for j in range(nq // ch):
            sl = slice(j * ch, (j + 1) * ch)
            nc.scalar.activation(
                u_sb[:, sl], t_sb[:, sl],
                mybir.ActivationFunctionType.Identity,
                bias=b_sb["bvs"], scale=b_sb["bvd"],
            )
        # t <- 1+tanh, tb <- 1-tanh (fused = 0.5*[o1*(1+t') + o2*(1-t')], 0.5 in wp)
        for j in range(nq // ch):
            sl = slice(j * ch, (j + 1) * ch)
            nc.gpsimd.tensor_scalar(
                tb_sb[:, sl], t_sb[:, sl], -1.0, 1.0,
                mybir.AluOpType.mult, mybir.AluOpType.add,
            )
            nc.gpsimd.tensor_scalar_add(t_sb[:, sl], t_sb[:, sl], 1.0)

        # Deferred thunks (GpSimd/DVE/PE-proj) drained inside attention loops.
        events = []

        mA = float(SCALE * A8)
        mB = float(B8 - SH * A8)

        def attention(a, o_sb, Q_sb, K_sb, post_chunk=None):
            for cidx in range(nch):
                p_out = pacc_pool.tile([C, ch], F32, tag="acc")
                p_den = pacc_pool.tile([C, ch], F32, tag="acc")
                pts = {}

                def scores(m):
                    # per-slice psum half-slots on a 4-deep ring: the
                    # producer's ~1.0us half-op round trip now fits inside
                    # the ~1.5us half-slot reuse distance, so the PE never
                    # waits on exp
                    ksl = slice(m * 128, (m + 1) * 128)
                    if m % 2 == 0:
                        pts[m // 2] = ppool.tile([C, 2, ch], E4, tag="pt", name="pt")
                    pt_half = pts[m // 2][:, m % 2, :]
                    for s in range(spc):
                        qsl = slice(cidx * ch + s * fd, cidx * ch + (s + 1) * fd)
                        ps = ps_pool.tile([C, fd], F32, tag="ps", name="pss")
                        nc.tensor.matmul(
                            ps[:], K_sb[:, ksl], Q_sb[:, qsl], start=True, stop=True
                        )
                        ssl = slice(s * fd, (s + 1) * fd)
                        if act_tile(m, mt):
                            nc.scalar.activation(
                                pt_half[:, ssl], ps[:],
                                mybir.ActivationFunctionType.Exp,
                                bias=b_sb["bsh"], scale=SCALE,
                            )
                        else:
                            nc.vector.tensor_scalar(
                                pt_half[:, ssl].bitcast(U8), ps[:], mA, mB,
                                mybir.AluOpType.mult, mybir.AluOpType.add,
                            )

                def pv_den(k):
                    # both PV and den contract key-tile pair k in one
                    # DoubleRow pass over the same [pt_2k|pt_2k+1] moving data
                    first, last = k == 0, k == mt // 2 - 1
                    wv = vpk[:, :, a, k * 128 : (k + 1) * 128]
                    for s in range(spc):
                        ssl = slice(s * fd, (s + 1) * fd)
                        nc.tensor.matmul(
                            p_out[:, ssl], wv, pts[k][:, :, ssl],
                            start=first, stop=last,
                            perf_mode=mybir.MatmulPerfMode.DoubleRow,
                        )
                    for s in range(spc):
                        ssl = slice(s * fd, (s + 1) * fd)
                        nc.tensor.matmul(
                            p_den[:, ssl], ones2[:], pts[k][:, :, ssl],
                            start=first, stop=last,
                            perf_mode=mybir.MatmulPerfMode.DoubleRow,
                        )
                    if k >= 2:
                        pts.pop(k - 2)

                # pv_den lags scores by 5 tiles (minimum 3): the first PV
                # of a chunk then fires after the previous chunk's normalize
                # has freed the accumulator slots
                scores(0)
                scores(1)
                for m in range(2, mt):
                    scores(m)
                    if m % 2 == 1 and m >= 5:
                        pv_den((m - 5) // 2)
                    elif m % 2 == 0 and events:
                        events.pop(0)()
                for k in (mt // 2 - 2, mt // 2 - 1):
                    pv_den(k)

                rec = rpool.tile([C, ch], F32, tag="rec")
                if a == 1 and cidx == nch - 1:
                    # tail chunk: halves let the fuse+projection chain start
                    # ~0.7us earlier
                    for s_ in range(spc):
                        ssl = slice(s_ * fd, (s_ + 1) * fd)
                        osl = slice(cidx * ch + s_ * fd, cidx * ch + (s_ + 1) * fd)
                        nc.vector.reciprocal_approx_fast(rec[:, ssl], p_den[:, ssl])
                        nc.vector.tensor_mul(o_sb[:, osl], p_out[:, ssl], rec[:, ssl])
                else:
                    nc.vector.reciprocal_approx_fast(rec[:], p_den[:])
                    osl = slice(cidx * ch, (cidx + 1) * ch)
                    nc.vector.tensor_mul(o_sb[:, osl], p_out[:], rec[:])

                if post_chunk is not None:
                    events.extend(post_chunk(cidx))

        attention(0, o1_sb, Q1_sb, K1_sb)

        # o1 <- o1*(1+t') runs as soon as attn1 finishes: drains during
        # attn2's early chunks, off the tail critical path
        for c in range(nch):
            for s_ in range(spc):
                sl = slice(c * ch + s_ * fd, c * ch + (s_ + 1) * fd)

                def _ga(sl=sl):
                    nc.vector.tensor_mul(o1_sb[:, sl], o1_sb[:, sl], t_sb[:, sl])

                events.append(_ga)

        def fuse_and_project(cidx):
            # out = wpTs.T @ [o1*(1+t') + o2*(1-t') + u] + bp, with the fuse
            # add and the u bias-term riding the projection's PSUM accumulation
            thunks = []
            for s in range(spc):
                sl = slice(cidx * ch + s * fd, cidx * ch + (s + 1) * fd)

                def _gb(sl=sl):
                    nc.vector.tensor_mul(d_sb[:, sl], o2_sb[:, sl], tb_sb[:, sl])

                def _proj(sl=sl):
                    ps = ps_pool.tile([C, fd], F32, tag="ps", name="psp")
                    nc.tensor.matmul(
                        ps[:], w_sb["wpTs"], o1_sb[:, sl], start=True, stop=False
                    )
                    nc.tensor.matmul(
                        ps[:], w_sb["wpTs"], d_sb[:, sl], start=False, stop=False
                    )
                    nc.tensor.matmul(
                        ps[:], w_sb["wpTs"], u_sb[:, sl], start=False, stop=True
                    )
                    st = stpool.tile([C, fd], F32, tag="st")
                    hq = fd // 2
                    for q_ in range(2):
                        qs = slice(q_ * hq, (q_ + 1) * hq)
                        nc.scalar.activation(
                            st[:, qs], ps[:, qs],
                            mybir.ActivationFunctionType.Identity, bias=b_sb["bp"],
                        )
                        osl = slice(sl.start + q_ * hq, sl.start + (q_ + 1) * hq)
                        nc.sync.dma_start(out=out_d[:, osl], in_=st[:, qs])

                thunks += [_gb, _proj]
            return thunks

        attention(1, o2_sb, Q2_sb, K2_sb, post_chunk=fuse_and_project)
        while events:
            events.pop(0)()

    nc.compile()
    return nc


def make_in_maps(hsi, msi, weights, n_cores=8):
    """Host-side sharding: core i handles (b=i//2, half=i%2); token axis rotated
    so the core's queries are columns [0, NQ)."""
    B = hsi.shape[0]
    hsi = np.ascontiguousarray(hsi.reshape(B, C, N_TOK), dtype=np.float16)
    msi = np.ascontiguousarray(msi.reshape(B, C, N_TOK), dtype=np.float16)
    in_maps = []
    for core in range(n_cores):
        b, h = core // 2, core % 2
        if h == 0:
            x_h, x_m = hsi[b], msi[b]
        else:
            x_h = np.concatenate([hsi[b][:, NQ:], hsi[b][:, :NQ]], axis=1)
            x_m = np.concatenate([msi[b][:, NQ:], msi[b][:, :NQ]], axis=1)
        m = {"x_h": np.ascontiguousarray(x_h), "x_m": np.ascontiguousarray(x_m)}
        m.update(weights)
        in_maps.append(m)
    return in_maps


def make_weight_map(
    wq1, bq1, wk1, bk1, wv1, bv1, wq2, bq2, wk2, bk2, wv2, bv2, wg, bg, wp, bp
):
    f = np.float32
    col = lambda v: np.ascontiguousarray(np.asarray(v, f).reshape(C, 1))
    tr = lambda w: np.ascontiguousarray(np.asarray(w, f).T)
    w = {
        "wq1T": tr(wq1), "wk1T": tr(wk1), "wq2T": tr(wq2), "wk2T": tr(wk2),
        "wgaT": tr(np.asarray(wg, f)[:, :C]),
        "wgbT": tr(np.asarray(wg, f)[:, C:]),
        "wv1T": tr(wv1),
        "wv2T": tr(wv2),
    }
    bv1a = np.asarray(bv1, f)
    bv2a = np.asarray(bv2, f)
    b = {
        "bq1": col(bq1), "bk1": col(bk1), "bq2": col(bq2), "bk2": col(bk2),
        "bvd": col(bv1a - bv2a), "bvs": col(bv1a + bv2a),
        "bgh": col(0.5 * np.asarray(bg, f)),
        "bp": col(bp), "bsh": col(np.full(C, -SH, f)),
    }
    wpackA = np.concatenate([w[n] for n, _ in WA], axis=1)
    wpackB = np.concatenate([w[n] for n, _ in WB], axis=1)
    bpack = np.concatenate([b[n] for n in BIAS_NAMES], axis=1)
    return {
        "wpackA": np.ascontiguousarray(wpackA.astype(np.float16)),
        "wpackB": np.ascontiguousarray(wpackB.astype(np.float16)),
        "wp32": np.ascontiguousarray(tr(0.5 * np.asarray(wp, f))),
        "bpack": np.ascontiguousarray(bpack),
    }


_NC_CACHE = {}


def _get_program():
    if "nc" not in _NC_CACHE:
        _NC_CACHE["nc"] = build_program()
    return _NC_CACHE["nc"]


def run_on_cores(in_maps, trace=False, **kwargs):
    from concourse.bass_utils import run_bass_kernel_spmd

    nc = _get_program()
    return run_bass_kernel_spmd(
        nc, in_maps, core_ids=list(range(len(in_maps))), trace=trace, **kwargs
    )


def kernel(
    hsi, msi, wq1, bq1, wk1, bk1, wv1, bv1, wq2, bq2, wk2, bk2, wv2, bv2,
    wg, bg, wp, bp,
):
    B, _, H, W = hsi.shape
    weights = make_weight_map(
        wq1, bq1, wk1, bk1, wv1, bv1, wq2, bq2, wk2, bk2, wv2, bv2, wg, bg, wp, bp
    )
    in_maps = make_in_maps(np.asarray(hsi), np.asarray(msi), weights)
    res = run_on_cores(in_maps)
    out = np.zeros((B, C, N_TOK), dtype=np.float32)
    for core in range(8):
        b, h = core // 2, core % 2
        out[b][:, h * NQ : (h + 1) * NQ] = res.results[core]["out"]
    return out.reshape(B, C, H, W)
